# revision 67
# baseline (speedup 1.0000x reference)
"""MLA forward on 8 Trainium2 NeuronCores — zero-collective design.

Each core owns one batch (4 cores per batch) and 512 query tokens arranged as
8 slots of 64, chosen so every core's causal attention has identical shape
(slot s attends 2s+2 key-tiles of 128). The kv path (compress + decompress of
all 16 heads for the full 2048 positions) is replicated across the batch's 4
cores, which removes all collectives from the critical path (only the tiny
latent AllGather remains, hidden under q-path compute). Output projection is
local to each core's tokens; the host reassembles.

fp16 everywhere: with random inputs the attention output is a diffuse average
(|out| ~ sigma_v/sqrt(k_eff)), so any per-element relative noise on the
q/k/v/p path passes ~1:1 to the final output — fp8 anywhere costs 3-9% rms
against a 2e-2 gate. Throughput instead comes from scheduling: weights are
prefetched ahead of their consumers, rope score matmuls are merged per
head-pair (N=128), and DMA queues are segregated so waiting transfers never
block compute-issuing queues.
"""

import numpy as np

import concourse.bacc as bacc
import concourse.mybir as mybir
import concourse.tile as tile
from concourse import bass_utils

B, S, D = 2, 2048, 2048
H = 16
NOPE, ROPE, VH = 128, 64, 128
HALF = ROPE // 2
QR = KVR = 512
EPS = 1e-6
W = 8                      # cores
LQ = 512                   # local q tokens per core
NSLOT = 8                  # q slots of 64
QW = 64
NKT = 16                   # key tiles of 128
SCALE = 1.0 / np.sqrt(NOPE + ROPE)
HPP = 8                    # heads per pass
NPASS = 2
# ownership: key tile b owned by core b%4; AG half t carries the owner's
# tile-slots {2t, 2t+1}; natural tile b sits at gathered block GPERM[b].
GPERM = [8 * ((b // 4) // 2) + 2 * (b % 4) + ((b // 4) % 2) for b in range(16)]

FP16 = mybir.dt.float16
FP32 = mybir.dt.float32
AF = mybir.ActivationFunctionType

_cache = {}


def _build():
    nc = bacc.Bacc("TRN2", target_bir_lowering=False, debug=False)

    def din(name, shape, dt=FP16):
        return nc.dram_tensor(name, shape, dt, kind="ExternalInput").ap()

    x_kv = din("x_kv", [128, 16, 512])   # this core's kv positions (p%4==j)
    x_q = din("x_q", [128, 16, LQ])
    w_cq = din("w_cq", [128, 16, QR])
    w_ckx = din("w_ckx", [128, 16, 640])      # ckv 512 | krope p2 64 | krope r2 64
    w_dqn = din("w_dqn", [128, 4, H * NOPE])
    w_dqr2 = din("w_dqr2", [128, 4, H * 128])  # per head-pair: p2|p2|r2|r2 blocks
    w_dkn = din("w_dkn", [128, 4, H * NOPE])
    w_dv = din("w_dv", [128, 4, H * VH])
    w_proj = din("w_proj", [128, 16, D])
    cs_kv = din("cs_kv", [64, 512])
    msc_kv = din("msc_kv", [64, 512])
    cs_q2 = din("cs_q2", [128, LQ])
    msc_q2 = din("msc_q2", [128, LQ])
    masks = din("masks", [128, 16, QW])             # additive 0/-10000 per slot/iter
    out_c = nc.dram_tensor("out_c", [128, 16, LQ], FP32, kind="ExternalOutput").ap()
    import os
    DBG = os.environ.get("KDBG") == "1"
    if DBG:
        dbg_qr2b = nc.dram_tensor("dbg_qr2b", [64, H, LQ], FP16, kind="ExternalOutput").ap()
        dbg_qn = nc.dram_tensor("dbg_qn", [128, H, LQ], FP16, kind="ExternalOutput").ap()
        dbg_qlat = nc.dram_tensor("dbg_qlat", [128, 4, LQ], FP16, kind="ExternalOutput").ap()
        dbg_kvl = nc.dram_tensor("dbg_kvl", [128, 4, S // 2], FP16, kind="ExternalOutput").ap()
        dbg_kn = nc.dram_tensor("dbg_kn", [128, HPP, S], FP16, kind="ExternalOutput").ap()
        dbg_ao = nc.dram_tensor("dbg_ao", [128, H, LQ], FP16, kind="ExternalOutput").ap()

    def contract(ps_ap, w_t, wc, wsl, r_t, rc, rsl, n):
        """Accumulate ps += sum over n chunks: w[:, c, wsl].T @ r[:, c, rsl]."""
        for i in range(n):
            nc.tensor.matmul(
                ps_ap, w_t[:, wc + i, wsl], r_t[:, rc + i, rsl],
                start=(i == 0), stop=(i == n - 1))

    cp_engines = None

    def cp(dst, src):
        cp_engines.append(cp_engines.pop(0))
        eng = cp_engines[-1]
        if eng is nc.scalar:
            eng.copy(dst, src)
        else:
            eng.tensor_copy(dst, src)

    with tile.TileContext(nc) as tc:
        cp_engines = [nc.vector, nc.scalar]

        const_cm = tc.tile_pool(name="const", bufs=1)
        const = const_cm.__enter__()
        zero1 = const.tile([128, 1], FP32, name="zero1")
        nc.any.memset(zero1[:], 0.0)
        eps_t = const.tile([1, 1], FP32, name="eps_t")
        nc.any.memset(eps_t[:], EPS)
        invn = const.tile([128, 1], FP16, name="invn")
        nc.any.memset(invn[:], 1.0 / QR)
        ones1 = const.tile([1, 128], FP16, name="ones1")
        nc.any.memset(ones1[:], 1.0)
        ones_h = const.tile([128, 128], FP16, name="ones_h")
        nc.any.memset(ones_h[:], 1.0)
        # persistent activations
        lat_cm = tc.tile_pool(name="lat", bufs=1)
        lat_pool = lat_cm.__enter__()
        kvlat_h = [lat_pool.tile([128, 4, S // 2], FP16, name=f"kvlat{t}")
                   for t in range(2)]
        qlat = lat_pool.tile([128, 4, LQ], FP16, name="qlat")
        krope_h = [lat_pool.tile([64, S // 2], FP16, name=f"krope{t}")
                   for t in range(2)]
        # k decompress weights: prefetched right after the compress inputs
        # (DMA issued inside the compress section); needed right after qdec
        wdkn_t = lat_pool.tile([128, 4, H * 128], FP16, name="wdkn_t")

        ap_cm = tc.tile_pool(name="att_persist", bufs=1)
        ap_pool = ap_cm.__enter__()
        attn_out = ap_pool.tile([128, H, LQ], FP16, name="attn_out")
        mask_t = ap_pool.tile([128, 16, QW], FP16, name="mask_t")
        qn_all = ap_pool.tile([128, H, LQ], FP16, name="qn_all")
        # attn_out is dead until pass-0 attention; alias its storage as the
        # qdec weight buffer so wdqn prefetches early with zero extra SBUF.
        # Heads 0-7 weights sit in the attn_out[:, 0:8] region (consumed
        # before pass 0 writes it); heads 8-15 weights in attn_out[:, 8:16]
        # (consumed at pass-0 s==4, written only by pass 1).
        wdqn_a = attn_out[:, 0:HPP, :].rearrange("p (c f) q -> p c (f q)", c=4)
        wdqn_b = attn_out[:, HPP:H, :].rearrange("p (c f) q -> p c (f q)", c=4)
        # all heads' rotated q rope combos in partitions 0:64, head-major
        qr2b = ap_pool.tile([64, H, LQ], FP16, name="qr2b")

        # ---------------- Phase A: compress own positions + AllGather --------
        # Each core compresses only its own 512 kv positions (p % 4 == j in
        # its batch), then the 4 batch cores AllGather the latents+krope in
        # two pipelined halves. The gathered loads undo the position
        # interleave so decompress sees natural key order.
        dram_cm = tc.tile_pool(name="dram", bufs=1, space="DRAM")
        dram = dram_cm.__enter__()
        lat_kin = [dram.tile([576, 256], FP16, tag=f"lat_kin{t}",
                             name=f"lat_kin{t}") for t in range(2)]
        lat_g = [dram.tile([4, 576, 256], FP16, tag=f"lat_g{t}",
                           name=f"lat_g{t}") for t in range(2)]

        with tc.tile_pool(name="cmp_x", bufs=1) as cmp_x, \
             tc.tile_pool(name="cmp_w", bufs=1) as cmp_w, \
             tc.tile_pool(name="cmp_t", bufs=2) as cmp_t, \
             tc.tile_pool(name="ps_cmp", bufs=1, space="PSUM") as ps_cmp, \
             tc.tile_pool(name="ps_nrm", bufs=1, space="PSUM") as ps_nrm:
            cs_kv_t = cmp_x.tile([64, 512], FP16, name="cs_kv_t")
            msc_kv_t = cmp_x.tile([64, 512], FP16, name="msc_kv_t")
            nc.gpsimd.dma_start(cs_kv_t[:], cs_kv[:])
            nc.gpsimd.dma_start(msc_kv_t[:], msc_kv[:])
            xkv_t = cmp_x.tile([128, 16, 512], FP16, name="xkv_t")
            xq_t = cmp_x.tile([128, 16, LQ], FP16, name="xq_t")
            wcq_t = cmp_w.tile([128, 16, QR], FP16, name="wcq_t")
            wckx_t = cmp_w.tile([128, 16, 640], FP16, name="wckx_t")
            # first chunks land fast (small DMAs) so the supertile can
            # start; the bulk follows as descriptor-cheap transfers
            nc.sync.dma_start(wckx_t[:, 0:1, :], w_ckx[:, 0:1, :])
            nc.sync.dma_start(xkv_t[:, 0:1, :], x_kv[:, 0:1, :])
            nc.sync.dma_start(wckx_t[:, 1:4, :], w_ckx[:, 1:4, :])
            nc.sync.dma_start(xkv_t[:, 1:4, :], x_kv[:, 1:4, :])
            nc.sync.dma_start(wckx_t[:, 4:16, :], w_ckx[:, 4:16, :])
            nc.sync.dma_start(xkv_t[:, 4:16, :], x_kv[:, 4:16, :])
            nc.sync.dma_start(wcq_t[:], w_cq[:])
            nc.sync.dma_start(xq_t[:], x_q[:])


            kr_sb = cmp_t.tile([64, 512], FP16, name="kr_sb", bufs=1)
            # supertile order kv0, kv1, q: both AGs ship back-to-back (the
            # collective device serializes them), q-path work then fills the
            # PE while they run
            for sup in (0, 1, 2):
                is_q = sup == 2
                xs = slice(0, LQ) if is_q else slice(sup * 256, (sup + 1) * 256)
                x_t = xq_t if is_q else xkv_t
                w_t = wcq_t if is_q else wckx_t
                nblk = 4 if is_q else 5
                nf = 512 if is_q else 256
                pss = [ps_cmp.tile([128, 512], FP32, name=f"ps_c{b}", tag=f"ps_c{b}")
                       for b in range(nblk)]
                for k in range(16):
                    for blk in range(nblk):
                        nc.tensor.matmul(
                            pss[blk][:, 0:nf], w_t[:, k, blk * 128:(blk + 1) * 128],
                            x_t[:, k, xs], start=(k == 0), stop=(k == 15))
                lat_raw = cmp_t.tile([128, 4, 512], FP16, name="lat_raw",
                                     tag="lat_raw", bufs=1)
                sq_t = cmp_t.tile([128, 4, 512], FP16, name="sq_t", tag="sq_t",
                                  bufs=1)
                for blk in range(4):
                    cp(lat_raw[:, blk, 0:nf], pss[blk][:, 0:nf])
                nc.vector.tensor_mul(sq_t[:, :, 0:nf], lat_raw[:, :, 0:nf],
                                     lat_raw[:, :, 0:nf])
                if not is_q:
                    # krope block: rows 0:64 p2, 64:128 r2 (cols 512:640)
                    u2 = cmp_t.tile([64, 512], FP16, name="u2", tag="u2")
                    v2 = cmp_t.tile([64, 512], FP16, name="v2", tag="v2")
                    nc.vector.tensor_mul(u2[:, 0:nf], pss[4][0:64, 0:nf],
                                         cs_kv_t[:, xs])
                    nc.vector.tensor_mul(v2[:, 0:nf], pss[4][64:128, 0:nf],
                                         msc_kv_t[:, xs])
                    nc.vector.tensor_add(kr_sb[:, xs], u2[:, 0:nf], v2[:, 0:nf])
                # rmsnorm: ssq -> rstd -> broadcast -> scale
                ps_ssq = ps_nrm.tile([1, 512], FP32, name="ps_ssq", tag="ps_ssq")
                for blk in range(4):
                    nc.tensor.matmul(ps_ssq[:, 0:nf], invn[:], sq_t[:, blk, 0:nf],
                                     start=(blk == 0), stop=(blk == 3))
                std_f = cmp_t.tile([1, 512], FP32, name="std_f", tag="std_f")
                nc.scalar.activation(std_f[:, 0:nf], ps_ssq[:, 0:nf], AF.Sqrt,
                                     bias=eps_t[:])
                rstd_f = cmp_t.tile([1, 512], FP16, name="rstd_f", tag="rstd_f")
                with nc.allow_low_precision(reason="rstd is O(1); fp16 ok"):
                    nc.vector.reciprocal(rstd_f[:, 0:nf], std_f[:, 0:nf])
                ps_rb = ps_nrm.tile([128, 512], FP32, name="ps_rb", tag="ps_rb")
                nc.tensor.matmul(ps_rb[:, 0:nf], ones1[:], rstd_f[:, 0:nf],
                                 start=True, stop=True)
                rstd_sb = cmp_t.tile([128, 512], FP16, name="rstd_sb", tag="rstd_sb")
                cp(rstd_sb[:, 0:nf], ps_rb[:, 0:nf])
                kvl_sb = cmp_t.tile([128, 4, 512], FP16, name="kvl_sb",
                                    tag="kvl_sb", bufs=1)
                if is_q:
                    nc.vector.tensor_mul(
                        qlat[:], lat_raw[:],
                        rstd_sb[:].unsqueeze(1).broadcast_to([128, 4, 512]))
                else:
                    nc.vector.tensor_mul(
                        kvl_sb[:, :, 0:nf], lat_raw[:, :, 0:nf],
                        rstd_sb[:, 0:nf].unsqueeze(1).broadcast_to([128, 4, nf]))
                    # ship this half: latent + krope -> DRAM -> AllGather.
                    # Stores ride the same in-order queue as the input loads
                    # so deferrable prefetches (emitted after) cannot take
                    # the serialized DMA device ahead of them.
                    t = sup
                    nc.sync.dma_start(
                        lat_kin[t][0:512, :].rearrange("(c p) n -> p c n", p=128),
                        kvl_sb[:, :, 0:nf])
                    nc.sync.dma_start(lat_kin[t][512:576, :], kr_sb[:, xs])
                    if t == 0:
                        nc.sync.dma_start(wdqn_a, w_dqn[:, :, 0:HPP * 128])
                        nc.sync.dma_start(wdkn_t[:], w_dkn[:])
                        nc.sync.dma_start(mask_t[:], masks[:])
                    nc.gpsimd.collective_compute(
                        "AllGather",
                        mybir.AluOpType.bypass,
                        ins=[lat_kin[t][:].rearrange("a b -> (a b)")],
                        outs=[lat_g[t][:].rearrange("w a b -> (w a b)")],
                        replica_groups=[[g * 4 + i for i in range(4)]
                                        for g in range(2)],
                    )


            # load gathered half 0 (gathered order; attention uses GPERM).
            # Half 1 is emitted inside pass 0 so its AG1-gated wait doesn't
            # coarsen into half-0 consumers.
            def load_gathered(t):
                for c in range(4):
                    nc.gpsimd.dma_start(
                        kvlat_h[t][:, c, :].rearrange("p (w m) -> p w m", w=4),
                        lat_g[t][:, c * 128:(c + 1) * 128, :]
                        .rearrange("w p m -> p w m"))
                nc.gpsimd.dma_start(
                    krope_h[t][:].rearrange("p (w m) -> p w m", w=4),
                    lat_g[t][:, 512:576, :].rearrange("w p m -> p w m"))

            load_gathered(0)

        # v decompress weights for both passes: pass-0 half issued here so it
        # lands during qdec; pass-1 half goes out during pass 0
        dw_cm = tc.tile_pool(name="dec_w", bufs=1)
        dec_w = dw_cm.__enter__()
        wdv_t = dec_w.tile([128, 4, H * 128], FP16, name="wdv_t")
        nc.sync.dma_start(wdv_t[:, :, 0:HPP * 128], w_dv[:, :, 0:HPP * 128])

        # remaining qdec weights: issued right after compress, land mid-qdec
        qdw2_cm = tc.tile_pool(name="qdec_w2", bufs=1)
        qdec_w2 = qdw2_cm.__enter__()
        wdqr2_t = qdec_w2.tile([128, 4, H * 128], FP16, name="wdqr2_t")
        cs_q2_t = qdec_w2.tile([128, LQ], FP16, name="cs_q2_t")
        msc_q2_t = qdec_w2.tile([128, LQ], FP16, name="msc_q2_t")
        nc.scalar.dma_start(wdqn_b, w_dqn[:, :, HPP * 128:H * 128])
        nc.scalar.dma_start(wdqr2_t[:], w_dqr2[:])
        nc.sync.dma_start(cs_q2_t[:], cs_q2[:])
        nc.sync.dma_start(msc_q2_t[:], msc_q2[:])

        # q decompress for all 16 heads up front (only needs qlat; fills the
        # PE while the latent AllGathers are in flight)
        with tc.tile_pool(name="qdec_t", bufs=2) as qdec_t, \
             tc.tile_pool(name="ps_qd", bufs=3, space="PSUM") as ps_qd:
            # heads 0-7 only; heads 8-15 (pass-1 consumers) decompress at
            # pass-0 s==4, filling the PE while AllGather-1 lands
            for h in range(HPP):
                ps = ps_qd.tile([128, 512], FP32, name="ps_qn", tag="ps_q")
                contract(ps[:], wdqn_a, 0, slice(h * 128, (h + 1) * 128),
                         qlat, 0, slice(0, LQ), 4)
                cp(qn_all[:, h, :], ps[:])
            for pr in range(H // 2):
                psp = ps_qd.tile([128, 512], FP32, name="ps_p2", tag="ps_q")
                psr = ps_qd.tile([128, 512], FP32, name="ps_r2", tag="ps_q")
                contract(psp[:], wdqr2_t, 0,
                         slice(pr * 256, pr * 256 + 128),
                         qlat, 0, slice(0, LQ), 4)
                contract(psr[:], wdqr2_t, 0,
                         slice(pr * 256 + 128, pr * 256 + 256),
                         qlat, 0, slice(0, LQ), 4)
                u2 = qdec_t.tile([128, 512], FP16, name="qu2", tag="qu2")
                v2 = qdec_t.tile([128, 512], FP16, name="qv2", tag="qv2")
                qtmp = qdec_t.tile([128, 512], FP16, name="qtmp", tag="qtmp")
                nc.vector.tensor_mul(u2[:], psp[:], cs_q2_t[:])
                nc.vector.tensor_mul(v2[:], psr[:], msc_q2_t[:])
                # head 2pr combo rows sit in partitions 0:64, head 2pr+1 in
                # 64:128; the odd head needs a partition-shifting DMA hop.
                nc.vector.tensor_add(qr2b[0:64, 2 * pr, :],
                                     u2[0:64, :], v2[0:64, :])
                nc.vector.tensor_add(qtmp[64:128, :],
                                     u2[64:128, :], v2[64:128, :])
                nc.scalar.dma_start(qr2b[0:64, 2 * pr + 1, :], qtmp[64:128, :])
        qdw2_cm.__exit__(None, None, None)

        # ---------------- Phase B: per head-pass decompress + attention ------
        for hp in range(NPASS):
            hbase = hp * HPP
            with tc.tile_pool(name="kv_sb", bufs=1) as kv_sb:
                psd_cm = tc.tile_pool(name="ps_dec", bufs=2, space="PSUM")
                ps_dec = psd_cm.__enter__()

                # --- kv decompress (8 heads, all 2048 keys) ---
                k_n = kv_sb.tile([128, HPP, S], FP16, name="k_n")
                v_t = kv_sb.tile([128, NKT, HPP * VH], FP16, name="v_t")

                def kvdec_half(tg):
                    lat = kvlat_h[tg]
                    for ksl in range(2):
                        ks = slice(ksl * 512, (ksl + 1) * 512)
                        ksg = slice((2 * tg + ksl) * 512, (2 * tg + ksl + 1) * 512)
                        for h in range(HPP):
                            ps = ps_dec.tile([128, 512], FP32, name="ps_kn",
                                             tag="ps_d")
                            contract(ps[:], wdkn_t, 0,
                                     slice((hbase + h) * 128,
                                           (hbase + h + 1) * 128),
                                     lat, 0, ks, 4)
                            cp(k_n[:, h, ksg], ps[:])
                    for ktl in range(8):
                        kt = 8 * tg + ktl
                        for g in range(2):      # head groups of 4
                            ps = ps_dec.tile([128, 512], FP32, name="ps_v",
                                             tag="ps_d")
                            for i in range(4):
                                nc.tensor.matmul(
                                    ps[:], lat[:, i, ktl * 128:(ktl + 1) * 128],
                                    wdv_t[:, i, hbase * 128 + g * 512:
                                          hbase * 128 + (g + 1) * 512],
                                    start=(i == 0), stop=(i == 3))
                            cp(v_t[:, kt, g * 512:(g + 1) * 512], ps[:])

                if hp == 0:
                    nc.sync.dma_start(wdv_t[:, :, HPP * 128:H * 128],
                                      w_dv[:, :, HPP * 128:H * 128])
                kvdec_half(0)
                ps_att_cm = tc.tile_pool(name="ps_att", bufs=2, space="PSUM")
                ps_att = ps_att_cm.__enter__()
                ps_avz_cm = tc.tile_pool(name="ps_avz", bufs=2, space="PSUM")
                ps_avz = ps_avz_cm.__enter__()
                att_cm = tc.tile_pool(name="att_t", bufs=2)
                att_t = att_cm.__enter__()

                # --- attention: 8 slots, slot s has 2s+2 key tiles ---
                # pav accumulation groups at F-offsets of one psum bank must
                # be sequential per head (interleaving breaks psum), so keep
                # the whole slot's exp'd probabilities in SBUF, then run each
                # head's AV as one contiguous accumulation group.
                def emit_av(st):
                    s_, pav_, pz_, pTs_ = st
                    trip_ = 2 * s_ + 2
                    qs_ = slice(s_ * QW, (s_ + 1) * QW)
                    for h in range(HPP):
                        fs = slice(h * QW, (h + 1) * QW)
                        vs = slice(h * VH, (h + 1) * VH)
                        for r in range(trip_):
                            nc.tensor.matmul(
                                pav_[:, fs], v_t[:, GPERM[r], vs],
                                pTs_[:, r, fs],
                                start=(r == 0), stop=(r == trip_ - 1))
                    rz = att_t.tile([128, 512], FP32, name="rz", tag="rz",
                                    bufs=1)
                    nc.vector.reciprocal(rz[:], pz_[:])
                    nc.vector.tensor_mul(
                        attn_out[:, hbase:hbase + HPP, qs_],
                        pav_[:].rearrange("p (h q) -> p h q", h=HPP),
                        rz[:].rearrange("p (h q) -> p h q", h=HPP))

                pend = None
                for s in range(NSLOT):
                    if s == 4:
                        if hp == 0:
                            # deferred q decompress (heads 8-15): useful PE
                            # work while AllGather-1 completes
                            for h in range(HPP, H):
                                ps = ps_dec.tile([128, 512], FP32,
                                                 name="ps_qn2", tag="ps_d")
                                contract(ps[:], wdqn_b, 0,
                                         slice((h - HPP) * 128,
                                               (h - HPP + 1) * 128),
                                         qlat, 0, slice(0, LQ), 4)
                                cp(qn_all[:, h, :], ps[:])
                            load_gathered(1)
                        kvdec_half(1)
                    trip = 2 * s + 2
                    qs = slice(s * QW, (s + 1) * QW)
                    pav = ps_avz.tile([128, 512], FP32, name="pav", tag="pav")
                    pz = ps_avz.tile([128, 512], FP32, name="pz", tag="pz",
                                     bufs=1)
                    pTs = att_t.tile([128, NKT, 512], FP16,
                                     name="pTs", tag="pTs", bufs=2)
                    for r in range(trip):
                        g = GPERM[r]
                        ks = slice(g * 128, (g + 1) * 128)
                        sc = ps_att.tile([128, 512], FP32, name="sc", tag="sc",
                                         bufs=3)
                        # per-head nope then rope: accumulation groups must
                        # be contiguous (the PE holds one open group; a new
                        # start discards any open accumulation)
                        for h in range(HPP):
                            fs = slice(h * QW, (h + 1) * QW)
                            nc.tensor.matmul(sc[:, fs], k_n[:, h, ks],
                                             qn_all[:, hbase + h, qs],
                                             start=True, stop=False)
                            nc.tensor.matmul(
                                sc[:, fs],
                                krope_h[g // 8][:, (g % 8) * 128:
                                                (g % 8) * 128 + 128],
                                qr2b[0:64, hbase + h, qs],
                                start=False, stop=True)
                        if r >= 2 * s:      # masked iters (diag + pad)
                            m = 2 * s + (r - 2 * s)
                            scv = sc[:].rearrange("p (h q) -> p h q", h=HPP)
                            mb = mask_t[:, m, :].unsqueeze(1).broadcast_to(
                                [128, HPP, QW])
                            nc.vector.tensor_add(scv, scv, mb)
                        nc.scalar.activation(pTs[:, r, :], sc[:], AF.Exp,
                                             bias=zero1[:])
                        if r == 1 and pend is not None:
                            emit_av(pend)
                            pend = None
                        # pz accumulates the full bank (safe to interleave
                        # with sc groups); deferred one iter so the PE never
                        # waits on the exp it just requested.
                        if r > 0:
                            nc.tensor.matmul(pz[:], ones_h[:], pTs[:, r - 1, :],
                                             start=(r == 1), stop=False)
                    nc.tensor.matmul(pz[:], ones_h[:], pTs[:, trip - 1, :],
                                     start=False, stop=True)
                    pend = (s, pav, pz, pTs)
                if pend is not None:
                    emit_av(pend)
                    pend = None
                att_cm.__exit__(None, None, None)
                ps_avz_cm.__exit__(None, None, None)
                ps_att_cm.__exit__(None, None, None)
                psd_cm.__exit__(None, None, None)

        dw_cm.__exit__(None, None, None)

        if DBG:
            nc.sync.dma_start(dbg_qr2b[:], qr2b[:])
            nc.sync.dma_start(dbg_qn[:], qn_all[:])
            nc.sync.dma_start(dbg_qlat[:], qlat[:])
            nc.sync.dma_start(dbg_kvl[:], kvlat_h[0][:])
            nc.sync.dma_start(dbg_ao[:], attn_out[:])

        # ---------------- Phase C: output projection -------------------------
        with tc.tile_pool(name="prj_w", bufs=3) as prj_w, \
             tc.tile_pool(name="prj_t", bufs=3) as prj_t, \
             tc.tile_pool(name="ps_prj", bufs=3, space="PSUM") as ps_prj:
            for ob in range(16):
                wp = prj_w.tile([128, 16, 128], FP16, name="wp", tag="wp")
                # w_proj is prepped ob-major: [:, ob, :] is one contiguous
                # 4KB/partition run (cheap descriptors)
                nc.gpsimd.dma_start(wp[:].rearrange("p a b -> p (a b)"),
                                    w_proj[:, ob, :])
                ps = ps_prj.tile([128, 512], FP32, name="ps_o", tag="ps_o")
                for h in range(16):
                    nc.tensor.matmul(ps[:], wp[:, h, :], attn_out[:, h, :],
                                     start=(h == 0), stop=(h == 15))
                ot = prj_t.tile([128, 512], FP32, name="ot", tag="ot")
                cp(ot[:], ps[:])
                nc.scalar.dma_start(out_c[:, ob, :], ot[:])

        ap_cm.__exit__(None, None, None)
        lat_cm.__exit__(None, None, None)
        dram_cm.__exit__(None, None, None)
        const_cm.__exit__(None, None, None)

    nc.compile()
    return nc


def _qsel(j):
    """Local q token order for core with within-batch index j."""
    idx = []
    for s in range(NSLOT):
        t = 4 * s + 3 - j
        idx.extend(range(t * QW, (t + 1) * QW))
    return np.array(idx)


def _prep_inputs(x, freqs_cis, w_cq, w_qnorm, w_dqn, w_dqr, w_ckv, w_kvnorm,
                 w_dkn, w_dv, w_krope, w_proj):
    f16 = np.float16

    perm = np.concatenate([np.arange(0, ROPE, 2), np.arange(1, ROPE, 2)])
    pe, po = perm[:HALF], perm[HALF:]

    def chunk_major(a, nch):
        # [K, C] -> [128, nch, C] with K = 128*nch
        return np.ascontiguousarray(
            a.reshape(nch, 128, a.shape[1]).transpose(1, 0, 2))

    # compress weights (lhsT layout [K=D, P=out])
    wcq_l = chunk_major(w_cq.T.astype(f16), 16)                   # [128,16,512]
    wkr = (w_krope / H)                                           # [64, D]
    ckx = np.concatenate([w_ckv, wkr[pe], wkr[pe], wkr[po], wkr[po]], axis=0)
    # krope block rows 512:640: p2 = [even;even], r2 = [odd;odd]
    wckx_l = chunk_major(ckx.T.astype(f16), 16)                   # [128,16,640]

    # decompress weights, norm + scale folded
    dqn = (w_dqn * w_qnorm[None, :] * SCALE)                      # [H*128, QR]
    wdqn_l = chunk_major(dqn.T.astype(f16), 4)                    # [128,4,2048]
    dqr = (w_dqr * w_qnorm[None, :] * SCALE).reshape(H, ROPE, QR)
    dqr2 = np.empty((H // 2, 4, HALF * 2, QR), np.float32)
    for p in range(H // 2):
        h0, h1 = 2 * p, 2 * p + 1
        # rows: [x0;x0] for p2 blocks, [x1;x1] for r2 blocks
        dqr2[p, 0, :HALF] = dqr[h0][pe]; dqr2[p, 0, HALF:] = dqr[h0][pe]
        dqr2[p, 1, :HALF] = dqr[h1][pe]; dqr2[p, 1, HALF:] = dqr[h1][pe]
        dqr2[p, 2, :HALF] = dqr[h0][po]; dqr2[p, 2, HALF:] = dqr[h0][po]
        dqr2[p, 3, :HALF] = dqr[h1][po]; dqr2[p, 3, HALF:] = dqr[h1][po]
    # layout per pair: cols [p2_h0(64) p2_h1(64) r2_h0(64) r2_h1(64)]
    dqr2 = dqr2.reshape(H // 2 * 4 * ROPE, QR)                    # [2048, 512]
    wdqr2_l = chunk_major(np.ascontiguousarray(dqr2.T).astype(f16), 4)
    dkn = (w_dkn * w_kvnorm[None, :])
    wdkn_l = chunk_major(dkn.T.astype(f16), 4)
    dvw = (w_dv * w_kvnorm[None, :])
    wdv_l = chunk_major(dvw.T.astype(f16), 4)
    # ob-major proj layout: [:, ob, :] = the 16 K-chunks x 128 d-cols of
    # output block ob, contiguous per partition
    wproj_l = chunk_major(np.ascontiguousarray(w_proj.T).astype(f16), 16)
    wproj_l = np.ascontiguousarray(
        wproj_l.reshape(128, 16, 16, 128).transpose(0, 2, 1, 3)
    ).reshape(128, 16, 2048)

    cos = freqs_cis[:, :, 0].T.astype(np.float32)                 # [32, S]
    sin = freqs_cis[:, :, 1].T.astype(np.float32)
    cs_kv = np.concatenate([cos, sin], 0).astype(f16)             # [64, S]
    msc_kv = np.concatenate([-sin, cos], 0).astype(f16)

    xT = [np.ascontiguousarray(x[b].T) for b in range(B)]         # [D, S]

    in_maps = []
    for c in range(W):
        b, j = c // 4, c % 4
        qsel = _qsel(j)
        kvsel = np.concatenate([np.arange(128 * (4 * k + j), 128 * (4 * k + j) + 128)
                                for k in range(4)])
        xkv_l = chunk_major(np.ascontiguousarray(xT[b][:, kvsel]).astype(f16), 16)
        xq_l = chunk_major(np.ascontiguousarray(xT[b][:, qsel]).astype(f16), 16)
        csq = cs_kv[:, qsel]
        mscq = msc_kv[:, qsel]
        cs_q2 = np.concatenate([csq, csq], 0)                     # [128, LQ]
        msc_q2 = np.concatenate([mscq, mscq], 0)
        # masks: slot s, d in {0,1} -> iter r = 2s+d, additive 0/-10000
        mk = np.zeros((128, 16, QW), np.float32)
        for s in range(NSLOT):
            for d_ in range(2):
                r = 2 * s + d_
                kg = r * 128 + np.arange(128)
                qg = qsel[s * QW:(s + 1) * QW]
                mk[:, 2 * s + d_, :] = np.where(qg[None, :] >= kg[:, None],
                                                0.0, -10000.0)
        in_maps.append({
            "x_kv": xkv_l, "x_q": xq_l,
            "w_cq": wcq_l, "w_ckx": wckx_l,
            "w_dqn": wdqn_l, "w_dqr2": wdqr2_l, "w_dkn": wdkn_l, "w_dv": wdv_l,
            "w_proj": wproj_l,
            "cs_kv": np.ascontiguousarray(cs_kv[:, kvsel]),
            "msc_kv": np.ascontiguousarray(msc_kv[:, kvsel]),
            "cs_q2": cs_q2.astype(f16), "msc_q2": msc_q2.astype(f16),
            "masks": mk.astype(f16),
        })
    return in_maps


last_results = None


def kernel(x, mask, freqs_cis, w_cq, w_qnorm, w_dqn, w_dqr, w_ckv, w_kvnorm,
           w_dkn, w_dv, w_krope, w_proj):
    global last_results
    if "nc" not in _cache:
        _cache["nc"] = _build()
    nc = _cache["nc"]

    args = [np.asarray(a, np.float32) for a in
            (x, freqs_cis, w_cq, w_qnorm, w_dqn, w_dqr, w_ckv, w_kvnorm,
             w_dkn, w_dv, w_krope, w_proj)]
    in_maps = _prep_inputs(*args)

    res = bass_utils.run_bass_kernel_spmd(nc, in_maps, core_ids=list(range(W)))
    last_results = res

    out = np.empty((B, S, D), np.float32)
    for c in range(W):
        b, j = c // 4, c % 4
        oc = res.results[c]["out_c"]          # [128, 16, 512]
        flat = oc.transpose(1, 0, 2).reshape(D, LQ)
        out[b, _qsel(j), :] = flat.T
    return out


# revision 69
# speedup vs baseline: 1.0231x; 1.0231x over previous
"""MLA forward on 8 Trainium2 NeuronCores — zero-collective design.

Each core owns one batch (4 cores per batch) and 512 query tokens arranged as
8 slots of 64, chosen so every core's causal attention has identical shape
(slot s attends 2s+2 key-tiles of 128). The kv path (compress + decompress of
all 16 heads for the full 2048 positions) is replicated across the batch's 4
cores, which removes all collectives from the critical path (only the tiny
latent AllGather remains, hidden under q-path compute). Output projection is
local to each core's tokens; the host reassembles.

fp16 everywhere: with random inputs the attention output is a diffuse average
(|out| ~ sigma_v/sqrt(k_eff)), so any per-element relative noise on the
q/k/v/p path passes ~1:1 to the final output — fp8 anywhere costs 3-9% rms
against a 2e-2 gate. Throughput instead comes from scheduling: weights are
prefetched ahead of their consumers, rope score matmuls are merged per
head-pair (N=128), and DMA queues are segregated so waiting transfers never
block compute-issuing queues.
"""

import numpy as np

import concourse.bacc as bacc
import concourse.mybir as mybir
import concourse.tile as tile
from concourse import bass_utils

B, S, D = 2, 2048, 2048
H = 16
NOPE, ROPE, VH = 128, 64, 128
HALF = ROPE // 2
QR = KVR = 512
EPS = 1e-6
W = 8                      # cores
LQ = 512                   # local q tokens per core
NSLOT = 8                  # q slots of 64
QW = 64
NKT = 16                   # key tiles of 128
SCALE = 1.0 / np.sqrt(NOPE + ROPE)
HPP = 8                    # heads per pass
NPASS = 2
# ownership: key tile b owned by core b%4; AG half t carries the owner's
# tile-slots {2t, 2t+1}; natural tile b sits at gathered block GPERM[b].
GPERM = [8 * ((b // 4) // 2) + 2 * (b % 4) + ((b // 4) % 2) for b in range(16)]

FP16 = mybir.dt.float16
FP32 = mybir.dt.float32
AF = mybir.ActivationFunctionType

_cache = {}


def _build():
    nc = bacc.Bacc("TRN2", target_bir_lowering=False, debug=False)

    def din(name, shape, dt=FP16):
        return nc.dram_tensor(name, shape, dt, kind="ExternalInput").ap()

    x_kv = din("x_kv", [128, 16, 512])   # this core's kv positions (p%4==j)
    x_q = din("x_q", [128, 16, LQ])
    w_cq = din("w_cq", [128, 16, QR])
    w_ckx = din("w_ckx", [128, 16, 640])      # ckv 512 | krope p2 64 | krope r2 64
    w_dqn = din("w_dqn", [128, 4, H * NOPE])
    w_dqr2 = din("w_dqr2", [128, 4, H * 128])  # per head-pair: p2|p2|r2|r2 blocks
    w_dkn = din("w_dkn", [128, 4, H * NOPE])
    w_dv = din("w_dv", [128, 4, H * VH])
    w_proj = din("w_proj", [128, 16, D])
    cs_kv = din("cs_kv", [64, 512])
    msc_kv = din("msc_kv", [64, 512])
    cs_q2 = din("cs_q2", [128, LQ])
    msc_q2 = din("msc_q2", [128, LQ])
    masks = din("masks", [128, 16, QW])             # additive 0/-10000 per slot/iter
    out_c = nc.dram_tensor("out_c", [128, 16, LQ], FP32, kind="ExternalOutput").ap()
    import os
    DBG = os.environ.get("KDBG") == "1"
    if DBG:
        dbg_qr2b = nc.dram_tensor("dbg_qr2b", [64, H, LQ], FP16, kind="ExternalOutput").ap()
        dbg_qn = nc.dram_tensor("dbg_qn", [128, H, LQ], FP16, kind="ExternalOutput").ap()
        dbg_qlat = nc.dram_tensor("dbg_qlat", [128, 4, LQ], FP16, kind="ExternalOutput").ap()
        dbg_kvl = nc.dram_tensor("dbg_kvl", [128, 4, S // 2], FP16, kind="ExternalOutput").ap()
        dbg_kn = nc.dram_tensor("dbg_kn", [128, HPP, S], FP16, kind="ExternalOutput").ap()
        dbg_ao = nc.dram_tensor("dbg_ao", [128, H, LQ], FP16, kind="ExternalOutput").ap()

    def contract(ps_ap, w_t, wc, wsl, r_t, rc, rsl, n):
        """Accumulate ps += sum over n chunks: w[:, c, wsl].T @ r[:, c, rsl]."""
        for i in range(n):
            nc.tensor.matmul(
                ps_ap, w_t[:, wc + i, wsl], r_t[:, rc + i, rsl],
                start=(i == 0), stop=(i == n - 1))

    cp_engines = None

    def cp(dst, src):
        cp_engines.append(cp_engines.pop(0))
        eng = cp_engines[-1]
        if eng is nc.scalar:
            eng.copy(dst, src)
        else:
            eng.tensor_copy(dst, src)

    with tile.TileContext(nc) as tc:
        cp_engines = [nc.vector, nc.scalar]

        const_cm = tc.tile_pool(name="const", bufs=1)
        const = const_cm.__enter__()
        zero1 = const.tile([128, 1], FP32, name="zero1")
        nc.any.memset(zero1[:], 0.0)
        eps_t = const.tile([1, 1], FP32, name="eps_t")
        nc.any.memset(eps_t[:], EPS)
        invn = const.tile([128, 1], FP16, name="invn")
        nc.any.memset(invn[:], 1.0 / QR)
        ones1 = const.tile([1, 128], FP16, name="ones1")
        nc.any.memset(ones1[:], 1.0)
        ones_h = const.tile([128, 128], FP16, name="ones_h")
        nc.any.memset(ones_h[:], 1.0)
        # persistent activations
        lat_cm = tc.tile_pool(name="lat", bufs=1)
        lat_pool = lat_cm.__enter__()
        kvlat_h = [lat_pool.tile([128, 4, S // 2], FP16, name=f"kvlat{t}")
                   for t in range(2)]
        qlat = lat_pool.tile([128, 4, LQ], FP16, name="qlat")
        krope_h = [lat_pool.tile([64, S // 2], FP16, name=f"krope{t}")
                   for t in range(2)]
        # k decompress weights: prefetched right after the compress inputs
        # (DMA issued inside the compress section); needed right after qdec
        wdkn_t = lat_pool.tile([128, 4, H * 128], FP16, name="wdkn_t")

        ap_cm = tc.tile_pool(name="att_persist", bufs=1)
        ap_pool = ap_cm.__enter__()
        attn_out = ap_pool.tile([128, H, LQ], FP16, name="attn_out")
        mask_t = ap_pool.tile([128, 16, QW], FP16, name="mask_t")
        qn_all = ap_pool.tile([128, H, LQ], FP16, name="qn_all")
        # attn_out is dead until pass-0 attention; alias its storage as the
        # qdec weight buffer so wdqn prefetches early with zero extra SBUF.
        # Heads 0-7 weights sit in the attn_out[:, 0:8] region (consumed
        # before pass 0 writes it); heads 8-15 weights in attn_out[:, 8:16]
        # (consumed at pass-0 s==4, written only by pass 1).
        wdqn_a = attn_out[:, 0:HPP, :].rearrange("p (c f) q -> p c (f q)", c=4)
        wdqn_b = attn_out[:, HPP:H, :].rearrange("p (c f) q -> p c (f q)", c=4)
        # all heads' rotated q rope combos in partitions 0:64, head-major
        qr2b = ap_pool.tile([64, H, LQ], FP16, name="qr2b")

        # ---------------- Phase A: compress own positions + AllGather --------
        # Each core compresses only its own 512 kv positions (p % 4 == j in
        # its batch), then the 4 batch cores AllGather the latents+krope in
        # two pipelined halves. The gathered loads undo the position
        # interleave so decompress sees natural key order.
        dram_cm = tc.tile_pool(name="dram", bufs=1, space="DRAM")
        dram = dram_cm.__enter__()
        lat_kin = [dram.tile([576, 256], FP16, tag=f"lat_kin{t}",
                             name=f"lat_kin{t}") for t in range(2)]
        lat_g = [dram.tile([4, 576, 256], FP16, tag=f"lat_g{t}",
                           name=f"lat_g{t}") for t in range(2)]

        with tc.tile_pool(name="cmp_x", bufs=1) as cmp_x, \
             tc.tile_pool(name="cmp_w", bufs=1) as cmp_w, \
             tc.tile_pool(name="cmp_t", bufs=2) as cmp_t, \
             tc.tile_pool(name="ps_cmp", bufs=1, space="PSUM") as ps_cmp, \
             tc.tile_pool(name="ps_nrm", bufs=1, space="PSUM") as ps_nrm:
            cs_kv_t = cmp_x.tile([64, 512], FP16, name="cs_kv_t")
            msc_kv_t = cmp_x.tile([64, 512], FP16, name="msc_kv_t")
            nc.gpsimd.dma_start(cs_kv_t[:], cs_kv[:])
            nc.gpsimd.dma_start(msc_kv_t[:], msc_kv[:])
            xkv_t = cmp_x.tile([128, 16, 512], FP16, name="xkv_t")
            xq_t = cmp_x.tile([128, 16, LQ], FP16, name="xq_t")
            wcq_t = cmp_w.tile([128, 16, QR], FP16, name="wcq_t")
            wckx_t = cmp_w.tile([128, 16, 640], FP16, name="wckx_t")
            # first chunks land fast (small DMAs) so the supertile can
            # start; the bulk follows as descriptor-cheap transfers
            nc.sync.dma_start(wckx_t[:, 0:1, :], w_ckx[:, 0:1, :])
            nc.sync.dma_start(xkv_t[:, 0:1, :], x_kv[:, 0:1, :])
            nc.sync.dma_start(wckx_t[:, 1:4, :], w_ckx[:, 1:4, :])
            nc.sync.dma_start(xkv_t[:, 1:4, :], x_kv[:, 1:4, :])
            nc.sync.dma_start(wckx_t[:, 4:10, :], w_ckx[:, 4:10, :])
            nc.sync.dma_start(xkv_t[:, 4:10, :], x_kv[:, 4:10, :])
            nc.sync.dma_start(wckx_t[:, 10:16, :], w_ckx[:, 10:16, :])
            nc.sync.dma_start(xkv_t[:, 10:16, :], x_kv[:, 10:16, :])
            nc.sync.dma_start(wcq_t[:], w_cq[:])
            nc.sync.dma_start(xq_t[:], x_q[:])


            kr_sb = cmp_t.tile([64, 512], FP16, name="kr_sb", bufs=1)
            # supertile order kv0, kv1, q: both AGs ship back-to-back (the
            # collective device serializes them), q-path work then fills the
            # PE while they run
            for sup in (0, 1, 2):
                is_q = sup == 2
                xs = slice(0, LQ) if is_q else slice(sup * 256, (sup + 1) * 256)
                x_t = xq_t if is_q else xkv_t
                w_t = wcq_t if is_q else wckx_t
                nblk = 4 if is_q else 5
                nf = 512 if is_q else 256
                pss = [ps_cmp.tile([128, 512], FP32, name=f"ps_c{b}", tag=f"ps_c{b}")
                       for b in range(nblk)]
                for k in range(16):
                    for blk in range(nblk):
                        nc.tensor.matmul(
                            pss[blk][:, 0:nf], w_t[:, k, blk * 128:(blk + 1) * 128],
                            x_t[:, k, xs], start=(k == 0), stop=(k == 15))
                lat_raw = cmp_t.tile([128, 4, 512], FP16, name="lat_raw",
                                     tag="lat_raw", bufs=1)
                sq_t = cmp_t.tile([128, 4, 512], FP16, name="sq_t", tag="sq_t",
                                  bufs=1)
                for blk in range(4):
                    cp(lat_raw[:, blk, 0:nf], pss[blk][:, 0:nf])
                nc.vector.tensor_mul(sq_t[:, :, 0:nf], lat_raw[:, :, 0:nf],
                                     lat_raw[:, :, 0:nf])
                if not is_q:
                    # krope block: rows 0:64 p2, 64:128 r2 (cols 512:640)
                    u2 = cmp_t.tile([64, 512], FP16, name="u2", tag="u2")
                    v2 = cmp_t.tile([64, 512], FP16, name="v2", tag="v2")
                    nc.vector.tensor_mul(u2[:, 0:nf], pss[4][0:64, 0:nf],
                                         cs_kv_t[:, xs])
                    nc.vector.tensor_mul(v2[:, 0:nf], pss[4][64:128, 0:nf],
                                         msc_kv_t[:, xs])
                    nc.vector.tensor_add(kr_sb[:, xs], u2[:, 0:nf], v2[:, 0:nf])
                # rmsnorm: ssq -> rstd -> broadcast -> scale
                ps_ssq = ps_nrm.tile([1, 512], FP32, name="ps_ssq", tag="ps_ssq")
                for blk in range(4):
                    nc.tensor.matmul(ps_ssq[:, 0:nf], invn[:], sq_t[:, blk, 0:nf],
                                     start=(blk == 0), stop=(blk == 3))
                std_f = cmp_t.tile([1, 512], FP32, name="std_f", tag="std_f")
                nc.scalar.activation(std_f[:, 0:nf], ps_ssq[:, 0:nf], AF.Sqrt,
                                     bias=eps_t[:])
                rstd_f = cmp_t.tile([1, 512], FP16, name="rstd_f", tag="rstd_f")
                with nc.allow_low_precision(reason="rstd is O(1); fp16 ok"):
                    nc.vector.reciprocal(rstd_f[:, 0:nf], std_f[:, 0:nf])
                ps_rb = ps_nrm.tile([128, 512], FP32, name="ps_rb", tag="ps_rb")
                nc.tensor.matmul(ps_rb[:, 0:nf], ones1[:], rstd_f[:, 0:nf],
                                 start=True, stop=True)
                rstd_sb = cmp_t.tile([128, 512], FP16, name="rstd_sb", tag="rstd_sb")
                cp(rstd_sb[:, 0:nf], ps_rb[:, 0:nf])
                kvl_sb = cmp_t.tile([128, 4, 512], FP16, name="kvl_sb",
                                    tag="kvl_sb", bufs=1)
                if is_q:
                    nc.vector.tensor_mul(
                        qlat[:], lat_raw[:],
                        rstd_sb[:].unsqueeze(1).broadcast_to([128, 4, 512]))
                else:
                    nc.vector.tensor_mul(
                        kvl_sb[:, :, 0:nf], lat_raw[:, :, 0:nf],
                        rstd_sb[:, 0:nf].unsqueeze(1).broadcast_to([128, 4, nf]))
                    # ship this half: latent + krope -> DRAM -> AllGather.
                    # Stores ride the same in-order queue as the input loads
                    # so deferrable prefetches (emitted after) cannot take
                    # the serialized DMA device ahead of them.
                    t = sup
                    nc.sync.dma_start(
                        lat_kin[t][0:512, :].rearrange("(c p) n -> p c n", p=128),
                        kvl_sb[:, :, 0:nf])
                    nc.sync.dma_start(lat_kin[t][512:576, :], kr_sb[:, xs])
                    if t == 0:
                        nc.sync.dma_start(wdqn_a, w_dqn[:, :, 0:HPP * 128])
                        nc.sync.dma_start(wdkn_t[:], w_dkn[:])
                        nc.sync.dma_start(mask_t[:], masks[:])
                    nc.gpsimd.collective_compute(
                        "AllGather",
                        mybir.AluOpType.bypass,
                        ins=[lat_kin[t][:].rearrange("a b -> (a b)")],
                        outs=[lat_g[t][:].rearrange("w a b -> (w a b)")],
                        replica_groups=[[g * 4 + i for i in range(4)]
                                        for g in range(2)],
                    )


            # load gathered half 0 (gathered order; attention uses GPERM).
            # Half 1 is emitted inside pass 0 so its AG1-gated wait doesn't
            # coarsen into half-0 consumers.
            def load_gathered(t):
                # first 512 keys (cols 0:512 = gathered blocks 0-1) of every
                # latent chunk land first so kvdec's ksl=0 can start early
                for cs_ in (slice(0, 512), slice(512, 1024)):
                    for c in range(4):
                        nc.gpsimd.dma_start(
                            kvlat_h[t][:, c, cs_].rearrange(
                                "p (w m) -> p w m", w=2),
                            lat_g[t][cs_.start // 256:cs_.stop // 256,
                                     c * 128:(c + 1) * 128, :]
                            .rearrange("w p m -> p w m"))
                nc.gpsimd.dma_start(
                    krope_h[t][:].rearrange("p (w m) -> p w m", w=4),
                    lat_g[t][:, 512:576, :].rearrange("w p m -> p w m"))

            load_gathered(0)

        # v decompress weights for both passes: pass-0 half issued here so it
        # lands during qdec; pass-1 half goes out during pass 0
        dw_cm = tc.tile_pool(name="dec_w", bufs=1)
        dec_w = dw_cm.__enter__()
        wdv_t = dec_w.tile([128, 4, H * 128], FP16, name="wdv_t")
        nc.sync.dma_start(wdv_t[:, :, 0:HPP * 128], w_dv[:, :, 0:HPP * 128])

        # remaining qdec weights: issued right after compress, land mid-qdec
        qdw2_cm = tc.tile_pool(name="qdec_w2", bufs=1)
        qdec_w2 = qdw2_cm.__enter__()
        wdqr2_t = qdec_w2.tile([128, 4, H * 128], FP16, name="wdqr2_t")
        cs_q2_t = qdec_w2.tile([128, LQ], FP16, name="cs_q2_t")
        msc_q2_t = qdec_w2.tile([128, LQ], FP16, name="msc_q2_t")
        nc.scalar.dma_start(wdqn_b, w_dqn[:, :, HPP * 128:H * 128])
        nc.scalar.dma_start(wdqr2_t[:], w_dqr2[:])
        nc.sync.dma_start(cs_q2_t[:], cs_q2[:])
        nc.sync.dma_start(msc_q2_t[:], msc_q2[:])

        # q decompress for all 16 heads up front (only needs qlat; fills the
        # PE while the latent AllGathers are in flight)
        with tc.tile_pool(name="qdec_t", bufs=2) as qdec_t, \
             tc.tile_pool(name="ps_qd", bufs=3, space="PSUM") as ps_qd:
            # heads 0-7 only; heads 8-15 (pass-1 consumers) decompress at
            # pass-0 s==4, filling the PE while AllGather-1 lands
            for h in range(HPP):
                ps = ps_qd.tile([128, 512], FP32, name="ps_qn", tag="ps_q")
                contract(ps[:], wdqn_a, 0, slice(h * 128, (h + 1) * 128),
                         qlat, 0, slice(0, LQ), 4)
                cp(qn_all[:, h, :], ps[:])
            for pr in range(H // 2):
                psp = ps_qd.tile([128, 512], FP32, name="ps_p2", tag="ps_q")
                psr = ps_qd.tile([128, 512], FP32, name="ps_r2", tag="ps_q")
                contract(psp[:], wdqr2_t, 0,
                         slice(pr * 256, pr * 256 + 128),
                         qlat, 0, slice(0, LQ), 4)
                contract(psr[:], wdqr2_t, 0,
                         slice(pr * 256 + 128, pr * 256 + 256),
                         qlat, 0, slice(0, LQ), 4)
                u2 = qdec_t.tile([128, 512], FP16, name="qu2", tag="qu2")
                v2 = qdec_t.tile([128, 512], FP16, name="qv2", tag="qv2")
                qtmp = qdec_t.tile([128, 512], FP16, name="qtmp", tag="qtmp")
                nc.vector.tensor_mul(u2[:], psp[:], cs_q2_t[:])
                nc.vector.tensor_mul(v2[:], psr[:], msc_q2_t[:])
                # head 2pr combo rows sit in partitions 0:64, head 2pr+1 in
                # 64:128; the odd head needs a partition-shifting DMA hop.
                nc.vector.tensor_add(qr2b[0:64, 2 * pr, :],
                                     u2[0:64, :], v2[0:64, :])
                nc.vector.tensor_add(qtmp[64:128, :],
                                     u2[64:128, :], v2[64:128, :])
                nc.scalar.dma_start(qr2b[0:64, 2 * pr + 1, :], qtmp[64:128, :])
        qdw2_cm.__exit__(None, None, None)

        # ---------------- Phase B: per head-pass decompress + attention ------
        for hp in range(NPASS):
            hbase = hp * HPP
            with tc.tile_pool(name="kv_sb", bufs=1) as kv_sb:
                psd_cm = tc.tile_pool(name="ps_dec", bufs=2, space="PSUM")
                ps_dec = psd_cm.__enter__()

                # --- kv decompress (8 heads, all 2048 keys) ---
                k_n = kv_sb.tile([128, HPP, S], FP16, name="k_n")
                v_t = kv_sb.tile([128, NKT, HPP * VH], FP16, name="v_t")

                def kvdec_half(tg):
                    lat = kvlat_h[tg]
                    for ksl in range(2):
                        ks = slice(ksl * 512, (ksl + 1) * 512)
                        ksg = slice((2 * tg + ksl) * 512, (2 * tg + ksl + 1) * 512)
                        for h in range(HPP):
                            ps = ps_dec.tile([128, 512], FP32, name="ps_kn",
                                             tag="ps_d")
                            contract(ps[:], wdkn_t, 0,
                                     slice((hbase + h) * 128,
                                           (hbase + h + 1) * 128),
                                     lat, 0, ks, 4)
                            cp(k_n[:, h, ksg], ps[:])
                    for ktl in range(8):
                        kt = 8 * tg + ktl
                        for g in range(2):      # head groups of 4
                            ps = ps_dec.tile([128, 512], FP32, name="ps_v",
                                             tag="ps_d")
                            for i in range(4):
                                nc.tensor.matmul(
                                    ps[:], lat[:, i, ktl * 128:(ktl + 1) * 128],
                                    wdv_t[:, i, hbase * 128 + g * 512:
                                          hbase * 128 + (g + 1) * 512],
                                    start=(i == 0), stop=(i == 3))
                            cp(v_t[:, kt, g * 512:(g + 1) * 512], ps[:])

                if hp == 0:
                    nc.sync.dma_start(wdv_t[:, :, HPP * 128:H * 128],
                                      w_dv[:, :, HPP * 128:H * 128])
                kvdec_half(0)
                ps_att_cm = tc.tile_pool(name="ps_att", bufs=2, space="PSUM")
                ps_att = ps_att_cm.__enter__()
                ps_avz_cm = tc.tile_pool(name="ps_avz", bufs=2, space="PSUM")
                ps_avz = ps_avz_cm.__enter__()
                att_cm = tc.tile_pool(name="att_t", bufs=2)
                att_t = att_cm.__enter__()

                # --- attention: 8 slots, slot s has 2s+2 key tiles ---
                # pav accumulation groups at F-offsets of one psum bank must
                # be sequential per head (interleaving breaks psum), so keep
                # the whole slot's exp'd probabilities in SBUF, then run each
                # head's AV as one contiguous accumulation group.
                def emit_av(st):
                    s_, pav_, pz_, pTs_ = st
                    trip_ = 2 * s_ + 2
                    qs_ = slice(s_ * QW, (s_ + 1) * QW)
                    for h in range(HPP):
                        fs = slice(h * QW, (h + 1) * QW)
                        vs = slice(h * VH, (h + 1) * VH)
                        for r in range(trip_):
                            nc.tensor.matmul(
                                pav_[:, fs], v_t[:, GPERM[r], vs],
                                pTs_[:, r, fs],
                                start=(r == 0), stop=(r == trip_ - 1))
                    rz = att_t.tile([128, 512], FP32, name="rz", tag="rz",
                                    bufs=1)
                    nc.vector.reciprocal(rz[:], pz_[:])
                    nc.vector.tensor_mul(
                        attn_out[:, hbase:hbase + HPP, qs_],
                        pav_[:].rearrange("p (h q) -> p h q", h=HPP),
                        rz[:].rearrange("p (h q) -> p h q", h=HPP))

                pend = None
                for s in range(NSLOT):
                    if s == 4:
                        if hp == 0:
                            # deferred q decompress (heads 8-15): useful PE
                            # work while AllGather-1 completes
                            for h in range(HPP, H):
                                ps = ps_dec.tile([128, 512], FP32,
                                                 name="ps_qn2", tag="ps_d")
                                contract(ps[:], wdqn_b, 0,
                                         slice((h - HPP) * 128,
                                               (h - HPP + 1) * 128),
                                         qlat, 0, slice(0, LQ), 4)
                                cp(qn_all[:, h, :], ps[:])
                            load_gathered(1)
                        kvdec_half(1)
                    trip = 2 * s + 2
                    qs = slice(s * QW, (s + 1) * QW)
                    pav = ps_avz.tile([128, 512], FP32, name="pav", tag="pav")
                    pz = ps_avz.tile([128, 512], FP32, name="pz", tag="pz",
                                     bufs=1)
                    pTs = att_t.tile([128, NKT, 512], FP16,
                                     name="pTs", tag="pTs", bufs=2)
                    for r in range(trip):
                        g = GPERM[r]
                        ks = slice(g * 128, (g + 1) * 128)
                        sc = ps_att.tile([128, 512], FP32, name="sc", tag="sc",
                                         bufs=3)
                        # per-head nope then rope: accumulation groups must
                        # be contiguous (the PE holds one open group; a new
                        # start discards any open accumulation)
                        for h in range(HPP):
                            fs = slice(h * QW, (h + 1) * QW)
                            nc.tensor.matmul(sc[:, fs], k_n[:, h, ks],
                                             qn_all[:, hbase + h, qs],
                                             start=True, stop=False)
                            nc.tensor.matmul(
                                sc[:, fs],
                                krope_h[g // 8][:, (g % 8) * 128:
                                                (g % 8) * 128 + 128],
                                qr2b[0:64, hbase + h, qs],
                                start=False, stop=True)
                        if r >= 2 * s:      # masked iters (diag + pad)
                            m = 2 * s + (r - 2 * s)
                            scv = sc[:].rearrange("p (h q) -> p h q", h=HPP)
                            mb = mask_t[:, m, :].unsqueeze(1).broadcast_to(
                                [128, HPP, QW])
                            nc.vector.tensor_add(scv, scv, mb)
                        nc.scalar.activation(pTs[:, r, :], sc[:], AF.Exp,
                                             bias=zero1[:])
                        if r == 1 and pend is not None:
                            emit_av(pend)
                            pend = None
                        # pz accumulates the full bank (safe to interleave
                        # with sc groups); deferred one iter so the PE never
                        # waits on the exp it just requested.
                        if r > 0:
                            nc.tensor.matmul(pz[:], ones_h[:], pTs[:, r - 1, :],
                                             start=(r == 1), stop=False)
                    nc.tensor.matmul(pz[:], ones_h[:], pTs[:, trip - 1, :],
                                     start=False, stop=True)
                    pend = (s, pav, pz, pTs)
                if pend is not None:
                    emit_av(pend)
                    pend = None
                att_cm.__exit__(None, None, None)
                ps_avz_cm.__exit__(None, None, None)
                ps_att_cm.__exit__(None, None, None)
                psd_cm.__exit__(None, None, None)

        dw_cm.__exit__(None, None, None)

        if DBG:
            nc.sync.dma_start(dbg_qr2b[:], qr2b[:])
            nc.sync.dma_start(dbg_qn[:], qn_all[:])
            nc.sync.dma_start(dbg_qlat[:], qlat[:])
            nc.sync.dma_start(dbg_kvl[:], kvlat_h[0][:])
            nc.sync.dma_start(dbg_ao[:], attn_out[:])

        # ---------------- Phase C: output projection -------------------------
        with tc.tile_pool(name="prj_w", bufs=3) as prj_w, \
             tc.tile_pool(name="prj_t", bufs=3) as prj_t, \
             tc.tile_pool(name="ps_prj", bufs=3, space="PSUM") as ps_prj:
            for ob in range(16):
                wp = prj_w.tile([128, 16, 128], FP16, name="wp", tag="wp")
                # w_proj is prepped ob-major: [:, ob, :] is one contiguous
                # 4KB/partition run (cheap descriptors)
                nc.gpsimd.dma_start(wp[:].rearrange("p a b -> p (a b)"),
                                    w_proj[:, ob, :])
                ps = ps_prj.tile([128, 512], FP32, name="ps_o", tag="ps_o")
                for h in range(16):
                    nc.tensor.matmul(ps[:], wp[:, h, :], attn_out[:, h, :],
                                     start=(h == 0), stop=(h == 15))
                ot = prj_t.tile([128, 512], FP32, name="ot", tag="ot")
                cp(ot[:], ps[:])
                nc.scalar.dma_start(out_c[:, ob, :], ot[:])

        ap_cm.__exit__(None, None, None)
        lat_cm.__exit__(None, None, None)
        dram_cm.__exit__(None, None, None)
        const_cm.__exit__(None, None, None)

    nc.compile()
    return nc


def _qsel(j):
    """Local q token order for core with within-batch index j."""
    idx = []
    for s in range(NSLOT):
        t = 4 * s + 3 - j
        idx.extend(range(t * QW, (t + 1) * QW))
    return np.array(idx)


def _prep_inputs(x, freqs_cis, w_cq, w_qnorm, w_dqn, w_dqr, w_ckv, w_kvnorm,
                 w_dkn, w_dv, w_krope, w_proj):
    f16 = np.float16

    perm = np.concatenate([np.arange(0, ROPE, 2), np.arange(1, ROPE, 2)])
    pe, po = perm[:HALF], perm[HALF:]

    def chunk_major(a, nch):
        # [K, C] -> [128, nch, C] with K = 128*nch
        return np.ascontiguousarray(
            a.reshape(nch, 128, a.shape[1]).transpose(1, 0, 2))

    # compress weights (lhsT layout [K=D, P=out])
    wcq_l = chunk_major(w_cq.T.astype(f16), 16)                   # [128,16,512]
    wkr = (w_krope / H)                                           # [64, D]
    ckx = np.concatenate([w_ckv, wkr[pe], wkr[pe], wkr[po], wkr[po]], axis=0)
    # krope block rows 512:640: p2 = [even;even], r2 = [odd;odd]
    wckx_l = chunk_major(ckx.T.astype(f16), 16)                   # [128,16,640]

    # decompress weights, norm + scale folded
    dqn = (w_dqn * w_qnorm[None, :] * SCALE)                      # [H*128, QR]
    wdqn_l = chunk_major(dqn.T.astype(f16), 4)                    # [128,4,2048]
    dqr = (w_dqr * w_qnorm[None, :] * SCALE).reshape(H, ROPE, QR)
    dqr2 = np.empty((H // 2, 4, HALF * 2, QR), np.float32)
    for p in range(H // 2):
        h0, h1 = 2 * p, 2 * p + 1
        # rows: [x0;x0] for p2 blocks, [x1;x1] for r2 blocks
        dqr2[p, 0, :HALF] = dqr[h0][pe]; dqr2[p, 0, HALF:] = dqr[h0][pe]
        dqr2[p, 1, :HALF] = dqr[h1][pe]; dqr2[p, 1, HALF:] = dqr[h1][pe]
        dqr2[p, 2, :HALF] = dqr[h0][po]; dqr2[p, 2, HALF:] = dqr[h0][po]
        dqr2[p, 3, :HALF] = dqr[h1][po]; dqr2[p, 3, HALF:] = dqr[h1][po]
    # layout per pair: cols [p2_h0(64) p2_h1(64) r2_h0(64) r2_h1(64)]
    dqr2 = dqr2.reshape(H // 2 * 4 * ROPE, QR)                    # [2048, 512]
    wdqr2_l = chunk_major(np.ascontiguousarray(dqr2.T).astype(f16), 4)
    dkn = (w_dkn * w_kvnorm[None, :])
    wdkn_l = chunk_major(dkn.T.astype(f16), 4)
    dvw = (w_dv * w_kvnorm[None, :])
    wdv_l = chunk_major(dvw.T.astype(f16), 4)
    # ob-major proj layout: [:, ob, :] = the 16 K-chunks x 128 d-cols of
    # output block ob, contiguous per partition
    wproj_l = chunk_major(np.ascontiguousarray(w_proj.T).astype(f16), 16)
    wproj_l = np.ascontiguousarray(
        wproj_l.reshape(128, 16, 16, 128).transpose(0, 2, 1, 3)
    ).reshape(128, 16, 2048)

    cos = freqs_cis[:, :, 0].T.astype(np.float32)                 # [32, S]
    sin = freqs_cis[:, :, 1].T.astype(np.float32)
    cs_kv = np.concatenate([cos, sin], 0).astype(f16)             # [64, S]
    msc_kv = np.concatenate([-sin, cos], 0).astype(f16)

    xT = [np.ascontiguousarray(x[b].T) for b in range(B)]         # [D, S]

    in_maps = []
    for c in range(W):
        b, j = c // 4, c % 4
        qsel = _qsel(j)
        kvsel = np.concatenate([np.arange(128 * (4 * k + j), 128 * (4 * k + j) + 128)
                                for k in range(4)])
        xkv_l = chunk_major(np.ascontiguousarray(xT[b][:, kvsel]).astype(f16), 16)
        xq_l = chunk_major(np.ascontiguousarray(xT[b][:, qsel]).astype(f16), 16)
        csq = cs_kv[:, qsel]
        mscq = msc_kv[:, qsel]
        cs_q2 = np.concatenate([csq, csq], 0)                     # [128, LQ]
        msc_q2 = np.concatenate([mscq, mscq], 0)
        # masks: slot s, d in {0,1} -> iter r = 2s+d, additive 0/-10000
        mk = np.zeros((128, 16, QW), np.float32)
        for s in range(NSLOT):
            for d_ in range(2):
                r = 2 * s + d_
                kg = r * 128 + np.arange(128)
                qg = qsel[s * QW:(s + 1) * QW]
                mk[:, 2 * s + d_, :] = np.where(qg[None, :] >= kg[:, None],
                                                0.0, -10000.0)
        in_maps.append({
            "x_kv": xkv_l, "x_q": xq_l,
            "w_cq": wcq_l, "w_ckx": wckx_l,
            "w_dqn": wdqn_l, "w_dqr2": wdqr2_l, "w_dkn": wdkn_l, "w_dv": wdv_l,
            "w_proj": wproj_l,
            "cs_kv": np.ascontiguousarray(cs_kv[:, kvsel]),
            "msc_kv": np.ascontiguousarray(msc_kv[:, kvsel]),
            "cs_q2": cs_q2.astype(f16), "msc_q2": msc_q2.astype(f16),
            "masks": mk.astype(f16),
        })
    return in_maps


last_results = None


def kernel(x, mask, freqs_cis, w_cq, w_qnorm, w_dqn, w_dqr, w_ckv, w_kvnorm,
           w_dkn, w_dv, w_krope, w_proj):
    global last_results
    if "nc" not in _cache:
        _cache["nc"] = _build()
    nc = _cache["nc"]

    args = [np.asarray(a, np.float32) for a in
            (x, freqs_cis, w_cq, w_qnorm, w_dqn, w_dqr, w_ckv, w_kvnorm,
             w_dkn, w_dv, w_krope, w_proj)]
    in_maps = _prep_inputs(*args)

    res = bass_utils.run_bass_kernel_spmd(nc, in_maps, core_ids=list(range(W)))
    last_results = res

    out = np.empty((B, S, D), np.float32)
    for c in range(W):
        b, j = c // 4, c % 4
        oc = res.results[c]["out_c"]          # [128, 16, 512]
        flat = oc.transpose(1, 0, 2).reshape(D, LQ)
        out[b, _qsel(j), :] = flat.T
    return out


# revision 74
# speedup vs baseline: 1.0482x; 1.0245x over previous
"""MLA forward on 8 Trainium2 NeuronCores — zero-collective design.

Each core owns one batch (4 cores per batch) and 512 query tokens arranged as
8 slots of 64, chosen so every core's causal attention has identical shape
(slot s attends 2s+2 key-tiles of 128). The kv path (compress + decompress of
all 16 heads for the full 2048 positions) is replicated across the batch's 4
cores, which removes all collectives from the critical path (only the tiny
latent AllGather remains, hidden under q-path compute). Output projection is
local to each core's tokens; the host reassembles.

fp16 everywhere: with random inputs the attention output is a diffuse average
(|out| ~ sigma_v/sqrt(k_eff)), so any per-element relative noise on the
q/k/v/p path passes ~1:1 to the final output — fp8 anywhere costs 3-9% rms
against a 2e-2 gate. Throughput instead comes from scheduling: weights are
prefetched ahead of their consumers, rope score matmuls are merged per
head-pair (N=128), and DMA queues are segregated so waiting transfers never
block compute-issuing queues.
"""

import numpy as np

import concourse.bacc as bacc
import concourse.mybir as mybir
import concourse.tile as tile
from concourse import bass_utils

B, S, D = 2, 2048, 2048
H = 16
NOPE, ROPE, VH = 128, 64, 128
HALF = ROPE // 2
QR = KVR = 512
EPS = 1e-6
W = 8                      # cores
LQ = 512                   # local q tokens per core
NSLOT = 8                  # q slots of 64
QW = 64
NKT = 16                   # key tiles of 128
SCALE = 1.0 / np.sqrt(NOPE + ROPE)
HPP = 8                    # heads per pass
NPASS = 2
# ownership: key tile b owned by core b%4; AG half t carries the owner's
# tile-slots {2t, 2t+1}; natural tile b sits at gathered block GPERM[b].
GPERM = [8 * ((b // 4) // 2) + 2 * (b % 4) + ((b // 4) % 2) for b in range(16)]

FP16 = mybir.dt.float16
FP32 = mybir.dt.float32
AF = mybir.ActivationFunctionType

_cache = {}


def _build():
    nc = bacc.Bacc("TRN2", target_bir_lowering=False, debug=False)

    def din(name, shape, dt=FP16):
        return nc.dram_tensor(name, shape, dt, kind="ExternalInput").ap()

    x_kv = din("x_kv", [128, 16, 512])   # this core's kv positions (p%4==j)
    x_q = din("x_q", [128, 16, LQ])
    w_cq = din("w_cq", [128, 16, QR])
    w_ckx = din("w_ckx", [128, 16, 640])      # ckv 512 | krope p2 64 | krope r2 64
    w_dqn = din("w_dqn", [128, 4, H * NOPE])
    w_dqr2 = din("w_dqr2", [128, 4, H * 128])  # per head-pair: p2|p2|r2|r2 blocks
    w_dkn = din("w_dkn", [128, 4, H * NOPE])
    w_dv = din("w_dv", [128, 4, H * VH])
    w_proj = din("w_proj", [128, 16, D])
    cs_kv = din("cs_kv", [64, 512])
    msc_kv = din("msc_kv", [64, 512])
    cs_q2 = din("cs_q2", [128, LQ])
    msc_q2 = din("msc_q2", [128, LQ])
    masks = din("masks", [128, 16, QW])             # additive 0/-10000 per slot/iter
    out_c = nc.dram_tensor("out_c", [128, 16, LQ], FP32, kind="ExternalOutput").ap()
    import os
    DBG = os.environ.get("KDBG") == "1"
    if DBG:
        dbg_qr2b = nc.dram_tensor("dbg_qr2b", [64, H, LQ], FP16, kind="ExternalOutput").ap()
        dbg_qn = nc.dram_tensor("dbg_qn", [128, H, LQ], FP16, kind="ExternalOutput").ap()
        dbg_qlat = nc.dram_tensor("dbg_qlat", [128, 4, LQ], FP16, kind="ExternalOutput").ap()
        dbg_kvl = nc.dram_tensor("dbg_kvl", [128, 4, S // 2], FP16, kind="ExternalOutput").ap()
        dbg_kn = nc.dram_tensor("dbg_kn", [128, HPP, S], FP16, kind="ExternalOutput").ap()
        dbg_ao = nc.dram_tensor("dbg_ao", [128, H, LQ], FP16, kind="ExternalOutput").ap()

    def contract(ps_ap, w_t, wc, wsl, r_t, rc, rsl, n):
        """Accumulate ps += sum over n chunks: w[:, c, wsl].T @ r[:, c, rsl]."""
        for i in range(n):
            nc.tensor.matmul(
                ps_ap, w_t[:, wc + i, wsl], r_t[:, rc + i, rsl],
                start=(i == 0), stop=(i == n - 1))

    cp_engines = None

    def cp(dst, src):
        cp_engines.append(cp_engines.pop(0))
        eng = cp_engines[-1]
        if eng is nc.scalar:
            eng.copy(dst, src)
        else:
            eng.tensor_copy(dst, src)

    with tile.TileContext(nc) as tc:
        cp_engines = [nc.vector, nc.scalar]

        const_cm = tc.tile_pool(name="const", bufs=1)
        const = const_cm.__enter__()
        zero1 = const.tile([128, 1], FP32, name="zero1")
        nc.any.memset(zero1[:], 0.0)
        eps_t = const.tile([1, 1], FP32, name="eps_t")
        nc.any.memset(eps_t[:], EPS)
        invn = const.tile([128, 1], FP16, name="invn")
        nc.any.memset(invn[:], 1.0 / QR)
        ones1 = const.tile([1, 128], FP16, name="ones1")
        nc.any.memset(ones1[:], 1.0)
        ones_h = const.tile([128, 128], FP16, name="ones_h")
        nc.any.memset(ones_h[:], 1.0)
        # persistent activations
        lat_cm = tc.tile_pool(name="lat", bufs=1)
        lat_pool = lat_cm.__enter__()
        kvlat_h = [lat_pool.tile([128, 4, S // 2], FP16, name=f"kvlat{t}")
                   for t in range(2)]
        qlat = lat_pool.tile([128, 4, LQ], FP16, name="qlat")
        krope_h = [lat_pool.tile([64, S // 2], FP16, name=f"krope{t}")
                   for t in range(2)]
        # k decompress weights: prefetched right after the compress inputs
        # (DMA issued inside the compress section); needed right after qdec
        wdkn_t = lat_pool.tile([128, 4, H * 128], FP16, name="wdkn_t")

        ap_cm = tc.tile_pool(name="att_persist", bufs=1)
        ap_pool = ap_cm.__enter__()
        attn_out = ap_pool.tile([128, H, LQ], FP16, name="attn_out")
        mask_t = ap_pool.tile([128, 16, QW], FP16, name="mask_t")
        qn_all = ap_pool.tile([128, H, LQ], FP16, name="qn_all")
        # attn_out is dead until pass-0 attention; alias its storage as the
        # qdec weight buffer so wdqn prefetches early with zero extra SBUF.
        # Heads 0-7 weights sit in the attn_out[:, 0:8] region (consumed
        # before pass 0 writes it); heads 8-15 weights in attn_out[:, 8:16]
        # (consumed at pass-0 s==4, written only by pass 1).
        wdqn_a = attn_out[:, 0:HPP, :].rearrange("p (c f) q -> p c (f q)", c=4)
        wdqn_b = attn_out[:, HPP:H, :].rearrange("p (c f) q -> p c (f q)", c=4)
        # all heads' rotated q rope combos in partitions 0:64, head-major
        qr2b = ap_pool.tile([64, H, LQ], FP16, name="qr2b")

        # ---------------- Phase A: compress own positions + AllGather --------
        # Each core compresses only its own 512 kv positions (p % 4 == j in
        # its batch), then the 4 batch cores AllGather the latents+krope in
        # two pipelined halves. The gathered loads undo the position
        # interleave so decompress sees natural key order.
        dram_cm = tc.tile_pool(name="dram", bufs=1, space="DRAM")
        dram = dram_cm.__enter__()
        lat_kin = [dram.tile([576, 256], FP16, tag=f"lat_kin{t}",
                             name=f"lat_kin{t}") for t in range(2)]
        lat_g = [dram.tile([4, 576, 256], FP16, tag=f"lat_g{t}",
                           name=f"lat_g{t}") for t in range(2)]

        with tc.tile_pool(name="cmp_x", bufs=1) as cmp_x, \
             tc.tile_pool(name="cmp_w", bufs=1) as cmp_w, \
             tc.tile_pool(name="cmp_t", bufs=2) as cmp_t, \
             tc.tile_pool(name="ps_cmp", bufs=1, space="PSUM") as ps_cmp, \
             tc.tile_pool(name="ps_nrm", bufs=1, space="PSUM") as ps_nrm:
            cs_kv_t = cmp_x.tile([64, 512], FP16, name="cs_kv_t")
            msc_kv_t = cmp_x.tile([64, 512], FP16, name="msc_kv_t")
            nc.gpsimd.dma_start(cs_kv_t[:], cs_kv[:])
            nc.gpsimd.dma_start(msc_kv_t[:], msc_kv[:])
            xkv_t = cmp_x.tile([128, 16, 512], FP16, name="xkv_t")
            xq_t = cmp_x.tile([128, 16, LQ], FP16, name="xq_t")
            wcq_t = cmp_w.tile([128, 16, QR], FP16, name="wcq_t")
            wckx_t = cmp_w.tile([128, 16, 640], FP16, name="wckx_t")
            # first chunks land fast (small DMAs) so the supertile can
            # start; the bulk follows as descriptor-cheap transfers.
            # x_kv loads column-split: kv0 (the AG0-critical supertile)
            # only reads cols 0:256, so its half ships first.
            nc.sync.dma_start(wckx_t[:, 0:1, :], w_ckx[:, 0:1, :])
            nc.sync.dma_start(xkv_t[:, 0:1, 0:256], x_kv[:, 0:1, 0:256])
            nc.sync.dma_start(wckx_t[:, 1:4, :], w_ckx[:, 1:4, :])
            nc.sync.dma_start(xkv_t[:, 1:4, 0:256], x_kv[:, 1:4, 0:256])
            nc.sync.dma_start(wckx_t[:, 4:10, :], w_ckx[:, 4:10, :])
            nc.sync.dma_start(xkv_t[:, 4:10, 0:256], x_kv[:, 4:10, 0:256])
            nc.sync.dma_start(wckx_t[:, 10:16, :], w_ckx[:, 10:16, :])
            nc.sync.dma_start(xkv_t[:, 10:16, 0:256], x_kv[:, 10:16, 0:256])
            nc.sync.dma_start(xkv_t[:, :, 256:512], x_kv[:, :, 256:512])
            nc.sync.dma_start(wcq_t[:], w_cq[:])
            nc.sync.dma_start(xq_t[:], x_q[:])


            kr_sb = cmp_t.tile([64, 512], FP16, name="kr_sb", bufs=1)
            # supertile order kv0, kv1, q: both AGs ship back-to-back (the
            # collective device serializes them), q-path work then fills the
            # PE while they run
            for sup in (0, 1, 2):
                is_q = sup == 2
                xs = slice(0, LQ) if is_q else slice(sup * 256, (sup + 1) * 256)
                x_t = xq_t if is_q else xkv_t
                w_t = wcq_t if is_q else wckx_t
                nblk = 4 if is_q else 5
                nf = 512 if is_q else 256
                pss = [ps_cmp.tile([128, 512], FP32, name=f"ps_c{b}", tag=f"ps_c{b}")
                       for b in range(nblk)]
                for k in range(16):
                    for blk in range(nblk):
                        nc.tensor.matmul(
                            pss[blk][:, 0:nf], w_t[:, k, blk * 128:(blk + 1) * 128],
                            x_t[:, k, xs], start=(k == 0), stop=(k == 15))
                lat_raw = cmp_t.tile([128, 4, 512], FP16, name="lat_raw",
                                     tag="lat_raw", bufs=1)
                sq_t = cmp_t.tile([128, 4, 512], FP16, name="sq_t", tag="sq_t",
                                  bufs=1)
                for blk in range(4):
                    cp(lat_raw[:, blk, 0:nf], pss[blk][:, 0:nf])
                nc.vector.tensor_mul(sq_t[:, :, 0:nf], lat_raw[:, :, 0:nf],
                                     lat_raw[:, :, 0:nf])
                if not is_q:
                    # krope block: rows 0:64 p2, 64:128 r2 (cols 512:640)
                    u2 = cmp_t.tile([64, 512], FP16, name="u2", tag="u2")
                    v2 = cmp_t.tile([64, 512], FP16, name="v2", tag="v2")
                    nc.vector.tensor_mul(u2[:, 0:nf], pss[4][0:64, 0:nf],
                                         cs_kv_t[:, xs])
                    nc.vector.tensor_mul(v2[:, 0:nf], pss[4][64:128, 0:nf],
                                         msc_kv_t[:, xs])
                    nc.vector.tensor_add(kr_sb[:, xs], u2[:, 0:nf], v2[:, 0:nf])
                # rmsnorm: ssq -> rstd -> broadcast -> scale
                ps_ssq = ps_nrm.tile([1, 512], FP32, name="ps_ssq", tag="ps_ssq")
                for blk in range(4):
                    nc.tensor.matmul(ps_ssq[:, 0:nf], invn[:], sq_t[:, blk, 0:nf],
                                     start=(blk == 0), stop=(blk == 3))
                std_f = cmp_t.tile([1, 512], FP32, name="std_f", tag="std_f")
                nc.scalar.activation(std_f[:, 0:nf], ps_ssq[:, 0:nf], AF.Sqrt,
                                     bias=eps_t[:])
                rstd_f = cmp_t.tile([1, 512], FP16, name="rstd_f", tag="rstd_f")
                with nc.allow_low_precision(reason="rstd is O(1); fp16 ok"):
                    nc.vector.reciprocal(rstd_f[:, 0:nf], std_f[:, 0:nf])
                ps_rb = ps_nrm.tile([128, 512], FP32, name="ps_rb", tag="ps_rb")
                nc.tensor.matmul(ps_rb[:, 0:nf], ones1[:], rstd_f[:, 0:nf],
                                 start=True, stop=True)
                rstd_sb = cmp_t.tile([128, 512], FP16, name="rstd_sb", tag="rstd_sb")
                cp(rstd_sb[:, 0:nf], ps_rb[:, 0:nf])
                kvl_sb = cmp_t.tile([128, 4, 512], FP16, name="kvl_sb",
                                    tag="kvl_sb", bufs=1)
                if is_q:
                    nc.vector.tensor_mul(
                        qlat[:], lat_raw[:],
                        rstd_sb[:].unsqueeze(1).broadcast_to([128, 4, 512]))
                else:
                    nc.vector.tensor_mul(
                        kvl_sb[:, :, 0:nf], lat_raw[:, :, 0:nf],
                        rstd_sb[:, 0:nf].unsqueeze(1).broadcast_to([128, 4, nf]))
                    # ship this half: latent + krope -> DRAM -> AllGather.
                    # Stores ride the same in-order queue as the input loads
                    # so deferrable prefetches (emitted after) cannot take
                    # the serialized DMA device ahead of them.
                    t = sup
                    nc.sync.dma_start(
                        lat_kin[t][0:512, :].rearrange("(c p) n -> p c n", p=128),
                        kvl_sb[:, :, 0:nf])
                    nc.sync.dma_start(lat_kin[t][512:576, :], kr_sb[:, xs])
                    if t == 0:
                        nc.sync.dma_start(wdqn_a, w_dqn[:, :, 0:HPP * 128])
                        nc.sync.dma_start(wdkn_t[:], w_dkn[:])
                        nc.sync.dma_start(mask_t[:], masks[:])
                    nc.gpsimd.collective_compute(
                        "AllGather",
                        mybir.AluOpType.bypass,
                        ins=[lat_kin[t][:].rearrange("a b -> (a b)")],
                        outs=[lat_g[t][:].rearrange("w a b -> (w a b)")],
                        replica_groups=[[g * 4 + i for i in range(4)]
                                        for g in range(2)],
                    )


            # load gathered half 0 (gathered order; attention uses GPERM).
            # Half 1 is emitted inside pass 0 so its AG1-gated wait doesn't
            # coarsen into half-0 consumers.
            def load_gathered(t):
                # first 512 keys (cols 0:512 = gathered blocks 0-1) of every
                # latent chunk land first so kvdec's ksl=0 can start early
                for cs_ in (slice(0, 512), slice(512, 1024)):
                    for c in range(4):
                        nc.gpsimd.dma_start(
                            kvlat_h[t][:, c, cs_].rearrange(
                                "p (w m) -> p w m", w=2),
                            lat_g[t][cs_.start // 256:cs_.stop // 256,
                                     c * 128:(c + 1) * 128, :]
                            .rearrange("w p m -> p w m"))
                nc.gpsimd.dma_start(
                    krope_h[t][:].rearrange("p (w m) -> p w m", w=4),
                    lat_g[t][:, 512:576, :].rearrange("w p m -> p w m"))

            load_gathered(0)

        # v decompress weights for both passes: pass-0 half issued here so it
        # lands during qdec; pass-1 half goes out during pass 0
        dw_cm = tc.tile_pool(name="dec_w", bufs=1)
        dec_w = dw_cm.__enter__()
        wdv_t = dec_w.tile([128, 4, H * 128], FP16, name="wdv_t")
        nc.sync.dma_start(wdv_t[:, :, 0:HPP * 128], w_dv[:, :, 0:HPP * 128])

        # remaining qdec weights: issued right after compress, land mid-qdec
        qdw2_cm = tc.tile_pool(name="qdec_w2", bufs=1)
        qdec_w2 = qdw2_cm.__enter__()
        wdqr2_t = qdec_w2.tile([128, 4, H * 128], FP16, name="wdqr2_t")
        cs_q2_t = qdec_w2.tile([128, LQ], FP16, name="cs_q2_t")
        msc_q2_t = qdec_w2.tile([128, LQ], FP16, name="msc_q2_t")
        nc.scalar.dma_start(wdqn_b, w_dqn[:, :, HPP * 128:H * 128])
        nc.scalar.dma_start(wdqr2_t[:], w_dqr2[:])
        nc.sync.dma_start(cs_q2_t[:], cs_q2[:])
        nc.sync.dma_start(msc_q2_t[:], msc_q2[:])

        # q decompress for all 16 heads up front (only needs qlat; fills the
        # PE while the latent AllGathers are in flight)
        with tc.tile_pool(name="qdec_t", bufs=2) as qdec_t, \
             tc.tile_pool(name="ps_qd", bufs=3, space="PSUM") as ps_qd:
            # heads 0-7 only; heads 8-15 (pass-1 consumers) decompress at
            # pass-0 s==4, filling the PE while AllGather-1 lands
            for h in range(HPP):
                ps = ps_qd.tile([128, 512], FP32, name="ps_qn", tag="ps_q")
                contract(ps[:], wdqn_a, 0, slice(h * 128, (h + 1) * 128),
                         qlat, 0, slice(0, LQ), 4)
                cp(qn_all[:, h, :], ps[:])
            for pr in range(H // 2):
                psp = ps_qd.tile([128, 512], FP32, name="ps_p2", tag="ps_q")
                psr = ps_qd.tile([128, 512], FP32, name="ps_r2", tag="ps_q")
                contract(psp[:], wdqr2_t, 0,
                         slice(pr * 256, pr * 256 + 128),
                         qlat, 0, slice(0, LQ), 4)
                contract(psr[:], wdqr2_t, 0,
                         slice(pr * 256 + 128, pr * 256 + 256),
                         qlat, 0, slice(0, LQ), 4)
                u2 = qdec_t.tile([128, 512], FP16, name="qu2", tag="qu2")
                v2 = qdec_t.tile([128, 512], FP16, name="qv2", tag="qv2")
                qtmp = qdec_t.tile([128, 512], FP16, name="qtmp", tag="qtmp")
                nc.vector.tensor_mul(u2[:], psp[:], cs_q2_t[:])
                nc.vector.tensor_mul(v2[:], psr[:], msc_q2_t[:])
                # head 2pr combo rows sit in partitions 0:64, head 2pr+1 in
                # 64:128; the odd head needs a partition-shifting DMA hop.
                nc.vector.tensor_add(qr2b[0:64, 2 * pr, :],
                                     u2[0:64, :], v2[0:64, :])
                nc.vector.tensor_add(qtmp[64:128, :],
                                     u2[64:128, :], v2[64:128, :])
                nc.scalar.dma_start(qr2b[0:64, 2 * pr + 1, :], qtmp[64:128, :])
        qdw2_cm.__exit__(None, None, None)

        # ---------------- Phase B: per head-pass decompress + attention ------
        for hp in range(NPASS):
            hbase = hp * HPP
            with tc.tile_pool(name="kv_sb", bufs=1) as kv_sb:
                psd_cm = tc.tile_pool(name="ps_dec", bufs=2, space="PSUM")
                ps_dec = psd_cm.__enter__()

                # --- kv decompress (8 heads, all 2048 keys) ---
                k_n = kv_sb.tile([128, HPP, S], FP16, name="k_n")
                v_t = kv_sb.tile([128, NKT, HPP * VH], FP16, name="v_t")

                def kvdec_half(tg):
                    lat = kvlat_h[tg]
                    for ksl in range(2):
                        ks = slice(ksl * 512, (ksl + 1) * 512)
                        ksg = slice((2 * tg + ksl) * 512, (2 * tg + ksl + 1) * 512)
                        for h in range(HPP):
                            ps = ps_dec.tile([128, 512], FP32, name="ps_kn",
                                             tag="ps_d")
                            contract(ps[:], wdkn_t, 0,
                                     slice((hbase + h) * 128,
                                           (hbase + h + 1) * 128),
                                     lat, 0, ks, 4)
                            cp(k_n[:, h, ksg], ps[:])
                    for ktl in range(8):
                        kt = 8 * tg + ktl
                        for g in range(2):      # head groups of 4
                            ps = ps_dec.tile([128, 512], FP32, name="ps_v",
                                             tag="ps_d")
                            for i in range(4):
                                nc.tensor.matmul(
                                    ps[:], lat[:, i, ktl * 128:(ktl + 1) * 128],
                                    wdv_t[:, i, hbase * 128 + g * 512:
                                          hbase * 128 + (g + 1) * 512],
                                    start=(i == 0), stop=(i == 3))
                            cp(v_t[:, kt, g * 512:(g + 1) * 512], ps[:])

                if hp == 0:
                    nc.sync.dma_start(wdv_t[:, :, HPP * 128:H * 128],
                                      w_dv[:, :, HPP * 128:H * 128])
                kvdec_half(0)
                ps_att_cm = tc.tile_pool(name="ps_att", bufs=2, space="PSUM")
                ps_att = ps_att_cm.__enter__()
                ps_avz_cm = tc.tile_pool(name="ps_avz", bufs=2, space="PSUM")
                ps_avz = ps_avz_cm.__enter__()
                att_cm = tc.tile_pool(name="att_t", bufs=2)
                att_t = att_cm.__enter__()

                # --- attention: 8 slots, slot s has 2s+2 key tiles ---
                # pav accumulation groups at F-offsets of one psum bank must
                # be sequential per head (interleaving breaks psum), so keep
                # the whole slot's exp'd probabilities in SBUF, then run each
                # head's AV as one contiguous accumulation group.
                def emit_av(st):
                    s_, pav_, pz_, pTs_ = st
                    trip_ = 2 * s_ + 2
                    qs_ = slice(s_ * QW, (s_ + 1) * QW)
                    for h in range(HPP):
                        fs = slice(h * QW, (h + 1) * QW)
                        vs = slice(h * VH, (h + 1) * VH)
                        for r in range(trip_):
                            nc.tensor.matmul(
                                pav_[:, fs], v_t[:, GPERM[r], vs],
                                pTs_[:, r, fs],
                                start=(r == 0), stop=(r == trip_ - 1))
                    rz = att_t.tile([128, 512], FP32, name="rz", tag="rz",
                                    bufs=1)
                    nc.vector.reciprocal(rz[:], pz_[:])
                    nc.vector.tensor_mul(
                        attn_out[:, hbase:hbase + HPP, qs_],
                        pav_[:].rearrange("p (h q) -> p h q", h=HPP),
                        rz[:].rearrange("p (h q) -> p h q", h=HPP))

                pend = None
                for s in range(NSLOT):
                    if s == 4:
                        if hp == 0:
                            # deferred q decompress (heads 8-15): useful PE
                            # work while AllGather-1 completes
                            for h in range(HPP, H):
                                ps = ps_dec.tile([128, 512], FP32,
                                                 name="ps_qn2", tag="ps_d")
                                contract(ps[:], wdqn_b, 0,
                                         slice((h - HPP) * 128,
                                               (h - HPP + 1) * 128),
                                         qlat, 0, slice(0, LQ), 4)
                                cp(qn_all[:, h, :], ps[:])
                            load_gathered(1)
                        kvdec_half(1)
                    trip = 2 * s + 2
                    qs = slice(s * QW, (s + 1) * QW)
                    pav = ps_avz.tile([128, 512], FP32, name="pav", tag="pav")
                    pz = ps_avz.tile([128, 512], FP32, name="pz", tag="pz",
                                     bufs=1)
                    pTs = att_t.tile([128, NKT, 512], FP16,
                                     name="pTs", tag="pTs", bufs=2)
                    for r in range(trip):
                        g = GPERM[r]
                        ks = slice(g * 128, (g + 1) * 128)
                        sc = ps_att.tile([128, 512], FP32, name="sc", tag="sc",
                                         bufs=3)
                        # per-head nope then rope: accumulation groups must
                        # be contiguous (the PE holds one open group; a new
                        # start discards any open accumulation)
                        for h in range(HPP):
                            fs = slice(h * QW, (h + 1) * QW)
                            nc.tensor.matmul(sc[:, fs], k_n[:, h, ks],
                                             qn_all[:, hbase + h, qs],
                                             start=True, stop=False)
                            nc.tensor.matmul(
                                sc[:, fs],
                                krope_h[g // 8][:, (g % 8) * 128:
                                                (g % 8) * 128 + 128],
                                qr2b[0:64, hbase + h, qs],
                                start=False, stop=True)
                        if r >= 2 * s:      # masked iters (diag + pad)
                            m = 2 * s + (r - 2 * s)
                            scv = sc[:].rearrange("p (h q) -> p h q", h=HPP)
                            mb = mask_t[:, m, :].unsqueeze(1).broadcast_to(
                                [128, HPP, QW])
                            nc.vector.tensor_add(scv, scv, mb)
                        nc.scalar.activation(pTs[:, r, :], sc[:], AF.Exp,
                                             bias=zero1[:])
                        if r == 1 and pend is not None:
                            emit_av(pend)
                            pend = None
                        # pz accumulates the full bank (safe to interleave
                        # with sc groups); deferred two iters so the PE never
                        # waits on in-flight exps.
                        if r > 1:
                            nc.tensor.matmul(pz[:], ones_h[:], pTs[:, r - 2, :],
                                             start=(r == 2), stop=False)
                    nc.tensor.matmul(pz[:], ones_h[:], pTs[:, trip - 2, :],
                                     start=(trip == 2), stop=False)
                    nc.tensor.matmul(pz[:], ones_h[:], pTs[:, trip - 1, :],
                                     start=False, stop=True)
                    pend = (s, pav, pz, pTs)
                if pend is not None:
                    emit_av(pend)
                    pend = None
                att_cm.__exit__(None, None, None)
                ps_avz_cm.__exit__(None, None, None)
                ps_att_cm.__exit__(None, None, None)
                psd_cm.__exit__(None, None, None)

        dw_cm.__exit__(None, None, None)

        if DBG:
            nc.sync.dma_start(dbg_qr2b[:], qr2b[:])
            nc.sync.dma_start(dbg_qn[:], qn_all[:])
            nc.sync.dma_start(dbg_qlat[:], qlat[:])
            nc.sync.dma_start(dbg_kvl[:], kvlat_h[0][:])
            nc.sync.dma_start(dbg_ao[:], attn_out[:])

        # ---------------- Phase C: output projection -------------------------
        with tc.tile_pool(name="prj_w", bufs=3) as prj_w, \
             tc.tile_pool(name="prj_t", bufs=3) as prj_t, \
             tc.tile_pool(name="ps_prj", bufs=3, space="PSUM") as ps_prj:
            for ob in range(16):
                wp = prj_w.tile([128, 16, 128], FP16, name="wp", tag="wp")
                # w_proj is prepped ob-major: [:, ob, :] is one contiguous
                # 4KB/partition run (cheap descriptors)
                nc.gpsimd.dma_start(wp[:].rearrange("p a b -> p (a b)"),
                                    w_proj[:, ob, :])
                ps = ps_prj.tile([128, 512], FP32, name="ps_o", tag="ps_o")
                for h in range(16):
                    nc.tensor.matmul(ps[:], wp[:, h, :], attn_out[:, h, :],
                                     start=(h == 0), stop=(h == 15))
                ot = prj_t.tile([128, 512], FP32, name="ot", tag="ot")
                cp(ot[:], ps[:])
                nc.scalar.dma_start(out_c[:, ob, :], ot[:])

        ap_cm.__exit__(None, None, None)
        lat_cm.__exit__(None, None, None)
        dram_cm.__exit__(None, None, None)
        const_cm.__exit__(None, None, None)

    nc.compile()
    return nc


def _qsel(j):
    """Local q token order for core with within-batch index j."""
    idx = []
    for s in range(NSLOT):
        t = 4 * s + 3 - j
        idx.extend(range(t * QW, (t + 1) * QW))
    return np.array(idx)


def _prep_inputs(x, freqs_cis, w_cq, w_qnorm, w_dqn, w_dqr, w_ckv, w_kvnorm,
                 w_dkn, w_dv, w_krope, w_proj):
    f16 = np.float16

    perm = np.concatenate([np.arange(0, ROPE, 2), np.arange(1, ROPE, 2)])
    pe, po = perm[:HALF], perm[HALF:]

    def chunk_major(a, nch):
        # [K, C] -> [128, nch, C] with K = 128*nch
        return np.ascontiguousarray(
            a.reshape(nch, 128, a.shape[1]).transpose(1, 0, 2))

    # compress weights (lhsT layout [K=D, P=out])
    wcq_l = chunk_major(w_cq.T.astype(f16), 16)                   # [128,16,512]
    wkr = (w_krope / H)                                           # [64, D]
    ckx = np.concatenate([w_ckv, wkr[pe], wkr[pe], wkr[po], wkr[po]], axis=0)
    # krope block rows 512:640: p2 = [even;even], r2 = [odd;odd]
    wckx_l = chunk_major(ckx.T.astype(f16), 16)                   # [128,16,640]

    # decompress weights, norm + scale folded
    dqn = (w_dqn * w_qnorm[None, :] * SCALE)                      # [H*128, QR]
    wdqn_l = chunk_major(dqn.T.astype(f16), 4)                    # [128,4,2048]
    dqr = (w_dqr * w_qnorm[None, :] * SCALE).reshape(H, ROPE, QR)
    dqr2 = np.empty((H // 2, 4, HALF * 2, QR), np.float32)
    for p in range(H // 2):
        h0, h1 = 2 * p, 2 * p + 1
        # rows: [x0;x0] for p2 blocks, [x1;x1] for r2 blocks
        dqr2[p, 0, :HALF] = dqr[h0][pe]; dqr2[p, 0, HALF:] = dqr[h0][pe]
        dqr2[p, 1, :HALF] = dqr[h1][pe]; dqr2[p, 1, HALF:] = dqr[h1][pe]
        dqr2[p, 2, :HALF] = dqr[h0][po]; dqr2[p, 2, HALF:] = dqr[h0][po]
        dqr2[p, 3, :HALF] = dqr[h1][po]; dqr2[p, 3, HALF:] = dqr[h1][po]
    # layout per pair: cols [p2_h0(64) p2_h1(64) r2_h0(64) r2_h1(64)]
    dqr2 = dqr2.reshape(H // 2 * 4 * ROPE, QR)                    # [2048, 512]
    wdqr2_l = chunk_major(np.ascontiguousarray(dqr2.T).astype(f16), 4)
    dkn = (w_dkn * w_kvnorm[None, :])
    wdkn_l = chunk_major(dkn.T.astype(f16), 4)
    dvw = (w_dv * w_kvnorm[None, :])
    wdv_l = chunk_major(dvw.T.astype(f16), 4)
    # ob-major proj layout: [:, ob, :] = the 16 K-chunks x 128 d-cols of
    # output block ob, contiguous per partition
    wproj_l = chunk_major(np.ascontiguousarray(w_proj.T).astype(f16), 16)
    wproj_l = np.ascontiguousarray(
        wproj_l.reshape(128, 16, 16, 128).transpose(0, 2, 1, 3)
    ).reshape(128, 16, 2048)

    cos = freqs_cis[:, :, 0].T.astype(np.float32)                 # [32, S]
    sin = freqs_cis[:, :, 1].T.astype(np.float32)
    cs_kv = np.concatenate([cos, sin], 0).astype(f16)             # [64, S]
    msc_kv = np.concatenate([-sin, cos], 0).astype(f16)

    xT = [np.ascontiguousarray(x[b].T) for b in range(B)]         # [D, S]

    in_maps = []
    for c in range(W):
        b, j = c // 4, c % 4
        qsel = _qsel(j)
        kvsel = np.concatenate([np.arange(128 * (4 * k + j), 128 * (4 * k + j) + 128)
                                for k in range(4)])
        xkv_l = chunk_major(np.ascontiguousarray(xT[b][:, kvsel]).astype(f16), 16)
        xq_l = chunk_major(np.ascontiguousarray(xT[b][:, qsel]).astype(f16), 16)
        csq = cs_kv[:, qsel]
        mscq = msc_kv[:, qsel]
        cs_q2 = np.concatenate([csq, csq], 0)                     # [128, LQ]
        msc_q2 = np.concatenate([mscq, mscq], 0)
        # masks: slot s, d in {0,1} -> iter r = 2s+d, additive 0/-10000
        mk = np.zeros((128, 16, QW), np.float32)
        for s in range(NSLOT):
            for d_ in range(2):
                r = 2 * s + d_
                kg = r * 128 + np.arange(128)
                qg = qsel[s * QW:(s + 1) * QW]
                mk[:, 2 * s + d_, :] = np.where(qg[None, :] >= kg[:, None],
                                                0.0, -10000.0)
        in_maps.append({
            "x_kv": xkv_l, "x_q": xq_l,
            "w_cq": wcq_l, "w_ckx": wckx_l,
            "w_dqn": wdqn_l, "w_dqr2": wdqr2_l, "w_dkn": wdkn_l, "w_dv": wdv_l,
            "w_proj": wproj_l,
            "cs_kv": np.ascontiguousarray(cs_kv[:, kvsel]),
            "msc_kv": np.ascontiguousarray(msc_kv[:, kvsel]),
            "cs_q2": cs_q2.astype(f16), "msc_q2": msc_q2.astype(f16),
            "masks": mk.astype(f16),
        })
    return in_maps


last_results = None


def kernel(x, mask, freqs_cis, w_cq, w_qnorm, w_dqn, w_dqr, w_ckv, w_kvnorm,
           w_dkn, w_dv, w_krope, w_proj):
    global last_results
    if "nc" not in _cache:
        _cache["nc"] = _build()
    nc = _cache["nc"]

    args = [np.asarray(a, np.float32) for a in
            (x, freqs_cis, w_cq, w_qnorm, w_dqn, w_dqr, w_ckv, w_kvnorm,
             w_dkn, w_dv, w_krope, w_proj)]
    in_maps = _prep_inputs(*args)

    res = bass_utils.run_bass_kernel_spmd(nc, in_maps, core_ids=list(range(W)))
    last_results = res

    out = np.empty((B, S, D), np.float32)
    for c in range(W):
        b, j = c // 4, c % 4
        oc = res.results[c]["out_c"]          # [128, 16, 512]
        flat = oc.transpose(1, 0, 2).reshape(D, LQ)
        out[b, _qsel(j), :] = flat.T
    return out


# revision 76
# speedup vs baseline: 1.0599x; 1.0112x over previous
"""MLA forward on 8 Trainium2 NeuronCores — zero-collective design.

Each core owns one batch (4 cores per batch) and 512 query tokens arranged as
8 slots of 64, chosen so every core's causal attention has identical shape
(slot s attends 2s+2 key-tiles of 128). The kv path (compress + decompress of
all 16 heads for the full 2048 positions) is replicated across the batch's 4
cores, which removes all collectives from the critical path (only the tiny
latent AllGather remains, hidden under q-path compute). Output projection is
local to each core's tokens; the host reassembles.

fp16 everywhere: with random inputs the attention output is a diffuse average
(|out| ~ sigma_v/sqrt(k_eff)), so any per-element relative noise on the
q/k/v/p path passes ~1:1 to the final output — fp8 anywhere costs 3-9% rms
against a 2e-2 gate. Throughput instead comes from scheduling: weights are
prefetched ahead of their consumers, rope score matmuls are merged per
head-pair (N=128), and DMA queues are segregated so waiting transfers never
block compute-issuing queues.
"""

import numpy as np

import concourse.bacc as bacc
import concourse.mybir as mybir
import concourse.tile as tile
from concourse import bass_utils

B, S, D = 2, 2048, 2048
H = 16
NOPE, ROPE, VH = 128, 64, 128
HALF = ROPE // 2
QR = KVR = 512
EPS = 1e-6
W = 8                      # cores
LQ = 512                   # local q tokens per core
NSLOT = 8                  # q slots of 64
QW = 64
NKT = 16                   # key tiles of 128
SCALE = 1.0 / np.sqrt(NOPE + ROPE)
HPP = 8                    # heads per pass
NPASS = 2
# ownership: key tile b owned by core b%4; AG half t carries the owner's
# tile-slots {2t, 2t+1}; natural tile b sits at gathered block GPERM[b].
GPERM = [8 * ((b // 4) // 2) + 2 * (b % 4) + ((b // 4) % 2) for b in range(16)]

FP16 = mybir.dt.float16
FP32 = mybir.dt.float32
AF = mybir.ActivationFunctionType

_cache = {}


def _build():
    nc = bacc.Bacc("TRN2", target_bir_lowering=False, debug=False)

    def din(name, shape, dt=FP16):
        return nc.dram_tensor(name, shape, dt, kind="ExternalInput").ap()

    x_kv = din("x_kv", [128, 16, 512])   # this core's kv positions (p%4==j)
    x_q = din("x_q", [128, 16, LQ])
    w_cq = din("w_cq", [128, 16, QR])
    w_ckx = din("w_ckx", [128, 16, 640])      # ckv 512 | krope p2 64 | krope r2 64
    w_dqn = din("w_dqn", [128, 4, H * NOPE])
    w_dqr2 = din("w_dqr2", [128, 4, H * 128])  # per head-pair: p2|p2|r2|r2 blocks
    w_dkn = din("w_dkn", [128, 4, H * NOPE])
    w_dv = din("w_dv", [128, 4, H * VH])
    w_proj = din("w_proj", [128, 16, D])
    cs_kv = din("cs_kv", [64, 512])
    msc_kv = din("msc_kv", [64, 512])
    cs_q2 = din("cs_q2", [128, LQ])
    msc_q2 = din("msc_q2", [128, LQ])
    masks = din("masks", [128, 16, QW])             # additive 0/-10000 per slot/iter
    out_c = nc.dram_tensor("out_c", [128, 16, LQ], FP32, kind="ExternalOutput").ap()
    import os
    DBG = os.environ.get("KDBG") == "1"
    if DBG:
        dbg_qr2b = nc.dram_tensor("dbg_qr2b", [64, H, LQ], FP16, kind="ExternalOutput").ap()
        dbg_qn = nc.dram_tensor("dbg_qn", [128, H, LQ], FP16, kind="ExternalOutput").ap()
        dbg_qlat = nc.dram_tensor("dbg_qlat", [128, 4, LQ], FP16, kind="ExternalOutput").ap()
        dbg_kvl = nc.dram_tensor("dbg_kvl", [128, 4, S // 2], FP16, kind="ExternalOutput").ap()
        dbg_kn = nc.dram_tensor("dbg_kn", [128, HPP, S], FP16, kind="ExternalOutput").ap()
        dbg_ao = nc.dram_tensor("dbg_ao", [128, H, LQ], FP16, kind="ExternalOutput").ap()

    def contract(ps_ap, w_t, wc, wsl, r_t, rc, rsl, n):
        """Accumulate ps += sum over n chunks: w[:, c, wsl].T @ r[:, c, rsl]."""
        for i in range(n):
            nc.tensor.matmul(
                ps_ap, w_t[:, wc + i, wsl], r_t[:, rc + i, rsl],
                start=(i == 0), stop=(i == n - 1))

    cp_engines = None

    def cp(dst, src):
        cp_engines.append(cp_engines.pop(0))
        eng = cp_engines[-1]
        if eng is nc.scalar:
            eng.copy(dst, src)
        else:
            eng.tensor_copy(dst, src)

    with tile.TileContext(nc) as tc:
        cp_engines = [nc.vector, nc.scalar]

        const_cm = tc.tile_pool(name="const", bufs=1)
        const = const_cm.__enter__()
        zero1 = const.tile([128, 1], FP32, name="zero1")
        nc.any.memset(zero1[:], 0.0)
        eps_t = const.tile([1, 1], FP32, name="eps_t")
        nc.any.memset(eps_t[:], EPS)
        invn = const.tile([128, 1], FP16, name="invn")
        nc.any.memset(invn[:], 1.0 / QR)
        ones1 = const.tile([1, 128], FP16, name="ones1")
        nc.any.memset(ones1[:], 1.0)
        ones_h = const.tile([128, 128], FP16, name="ones_h")
        nc.any.memset(ones_h[:], 1.0)
        # persistent activations
        lat_cm = tc.tile_pool(name="lat", bufs=1)
        lat_pool = lat_cm.__enter__()
        kvlat_h = [lat_pool.tile([128, 4, S // 2], FP16, name=f"kvlat{t}")
                   for t in range(2)]
        qlat = lat_pool.tile([128, 4, LQ], FP16, name="qlat")
        krope_h = [lat_pool.tile([64, S // 2], FP16, name=f"krope{t}")
                   for t in range(2)]
        # k decompress weights: prefetched right after the compress inputs
        # (DMA issued inside the compress section); needed right after qdec
        wdkn_t = lat_pool.tile([128, 4, H * 128], FP16, name="wdkn_t")

        ap_cm = tc.tile_pool(name="att_persist", bufs=1)
        ap_pool = ap_cm.__enter__()
        attn_out = ap_pool.tile([128, H, LQ], FP16, name="attn_out")
        mask_t = ap_pool.tile([128, 16, QW], FP16, name="mask_t")
        qn_all = ap_pool.tile([128, H, LQ], FP16, name="qn_all")
        # attn_out is dead until pass-0 attention; alias its storage as the
        # qdec weight buffer so wdqn prefetches early with zero extra SBUF.
        # Heads 0-7 weights sit in the attn_out[:, 0:8] region (consumed
        # before pass 0 writes it); heads 8-15 weights in attn_out[:, 8:16]
        # (consumed at pass-0 s==4, written only by pass 1).
        wdqn_a = attn_out[:, 0:HPP, :].rearrange("p (c f) q -> p c (f q)", c=4)
        wdqn_b = attn_out[:, HPP:H, :].rearrange("p (c f) q -> p c (f q)", c=4)
        # all heads' rotated q rope combos in partitions 0:64, head-major
        qr2b = ap_pool.tile([64, H, LQ], FP16, name="qr2b")

        # ---------------- Phase A: compress own positions + AllGather --------
        # Each core compresses only its own 512 kv positions (p % 4 == j in
        # its batch), then the 4 batch cores AllGather the latents+krope in
        # two pipelined halves. The gathered loads undo the position
        # interleave so decompress sees natural key order.
        dram_cm = tc.tile_pool(name="dram", bufs=1, space="DRAM")
        dram = dram_cm.__enter__()
        lat_kin = [dram.tile([576, 256], FP16, tag=f"lat_kin{t}",
                             name=f"lat_kin{t}") for t in range(2)]
        lat_g = [dram.tile([4, 576, 256], FP16, tag=f"lat_g{t}",
                           name=f"lat_g{t}") for t in range(2)]

        with tc.tile_pool(name="cmp_x", bufs=1) as cmp_x, \
             tc.tile_pool(name="cmp_w", bufs=1) as cmp_w, \
             tc.tile_pool(name="cmp_t", bufs=2) as cmp_t, \
             tc.tile_pool(name="ps_cmp", bufs=1, space="PSUM") as ps_cmp, \
             tc.tile_pool(name="ps_nrm", bufs=1, space="PSUM") as ps_nrm:
            cs_kv_t = cmp_x.tile([64, 512], FP16, name="cs_kv_t")
            msc_kv_t = cmp_x.tile([64, 512], FP16, name="msc_kv_t")
            nc.gpsimd.dma_start(cs_kv_t[:], cs_kv[:])
            nc.gpsimd.dma_start(msc_kv_t[:], msc_kv[:])
            xkv_t = cmp_x.tile([128, 16, 512], FP16, name="xkv_t")
            xq_t = cmp_x.tile([128, 16, LQ], FP16, name="xq_t")
            wcq_t = cmp_w.tile([128, 16, QR], FP16, name="wcq_t")
            wckx_t = cmp_w.tile([128, 16, 640], FP16, name="wckx_t")
            # first chunks land fast (small DMAs) so the supertile can
            # start; the bulk follows as descriptor-cheap transfers.
            # x_kv loads column-split: kv0 (the AG0-critical supertile)
            # only reads cols 0:256, so its half ships first.
            nc.sync.dma_start(wckx_t[:, 0:1, :], w_ckx[:, 0:1, :])
            nc.sync.dma_start(xkv_t[:, 0:1, 0:256], x_kv[:, 0:1, 0:256])
            nc.sync.dma_start(wckx_t[:, 1:4, :], w_ckx[:, 1:4, :])
            nc.sync.dma_start(xkv_t[:, 1:4, 0:256], x_kv[:, 1:4, 0:256])
            nc.sync.dma_start(wckx_t[:, 4:10, :], w_ckx[:, 4:10, :])
            nc.sync.dma_start(xkv_t[:, 4:10, 0:256], x_kv[:, 4:10, 0:256])
            nc.sync.dma_start(wckx_t[:, 10:16, :], w_ckx[:, 10:16, :])
            nc.sync.dma_start(xkv_t[:, 10:16, 0:256], x_kv[:, 10:16, 0:256])
            nc.sync.dma_start(xkv_t[:, :, 256:512], x_kv[:, :, 256:512])
            nc.sync.dma_start(wcq_t[:], w_cq[:])
            nc.sync.dma_start(xq_t[:], x_q[:])


            kr_sb = cmp_t.tile([64, 512], FP16, name="kr_sb", bufs=1)
            # supertile order kv0, kv1, q: both AGs ship back-to-back (the
            # collective device serializes them), q-path work then fills the
            # PE while they run
            for sup in (0, 1, 2):
                is_q = sup == 2
                xs = slice(0, LQ) if is_q else slice(sup * 256, (sup + 1) * 256)
                x_t = xq_t if is_q else xkv_t
                w_t = wcq_t if is_q else wckx_t
                nblk = 4 if is_q else 5
                nf = 512 if is_q else 256
                pss = [ps_cmp.tile([128, 512], FP32, name=f"ps_c{b}", tag=f"ps_c{b}")
                       for b in range(nblk)]
                for k in range(16):
                    for blk in range(nblk):
                        nc.tensor.matmul(
                            pss[blk][:, 0:nf], w_t[:, k, blk * 128:(blk + 1) * 128],
                            x_t[:, k, xs], start=(k == 0), stop=(k == 15))
                lat_raw = cmp_t.tile([128, 4, 512], FP16, name="lat_raw",
                                     tag="lat_raw", bufs=1)
                sq_t = cmp_t.tile([128, 4, 512], FP16, name="sq_t", tag="sq_t",
                                  bufs=1)
                for blk in range(4):
                    cp(lat_raw[:, blk, 0:nf], pss[blk][:, 0:nf])
                nc.vector.tensor_mul(sq_t[:, :, 0:nf], lat_raw[:, :, 0:nf],
                                     lat_raw[:, :, 0:nf])
                if not is_q:
                    # krope block: rows 0:64 p2, 64:128 r2 (cols 512:640)
                    u2 = cmp_t.tile([64, 512], FP16, name="u2", tag="u2")
                    v2 = cmp_t.tile([64, 512], FP16, name="v2", tag="v2")
                    nc.vector.tensor_mul(u2[:, 0:nf], pss[4][0:64, 0:nf],
                                         cs_kv_t[:, xs])
                    nc.vector.tensor_mul(v2[:, 0:nf], pss[4][64:128, 0:nf],
                                         msc_kv_t[:, xs])
                    nc.vector.tensor_add(kr_sb[:, xs], u2[:, 0:nf], v2[:, 0:nf])
                # rmsnorm: ssq -> rstd -> broadcast -> scale
                ps_ssq = ps_nrm.tile([1, 512], FP32, name="ps_ssq", tag="ps_ssq")
                for blk in range(4):
                    nc.tensor.matmul(ps_ssq[:, 0:nf], invn[:], sq_t[:, blk, 0:nf],
                                     start=(blk == 0), stop=(blk == 3))
                std_f = cmp_t.tile([1, 512], FP32, name="std_f", tag="std_f")
                nc.scalar.activation(std_f[:, 0:nf], ps_ssq[:, 0:nf], AF.Sqrt,
                                     bias=eps_t[:])
                rstd_f = cmp_t.tile([1, 512], FP16, name="rstd_f", tag="rstd_f")
                with nc.allow_low_precision(reason="rstd is O(1); fp16 ok"):
                    nc.vector.reciprocal(rstd_f[:, 0:nf], std_f[:, 0:nf])
                ps_rb = ps_nrm.tile([128, 512], FP32, name="ps_rb", tag="ps_rb")
                nc.tensor.matmul(ps_rb[:, 0:nf], ones1[:], rstd_f[:, 0:nf],
                                 start=True, stop=True)
                rstd_sb = cmp_t.tile([128, 512], FP16, name="rstd_sb", tag="rstd_sb")
                cp(rstd_sb[:, 0:nf], ps_rb[:, 0:nf])
                kvl_sb = cmp_t.tile([128, 4, 512], FP16, name="kvl_sb",
                                    tag="kvl_sb", bufs=1)
                if is_q:
                    nc.vector.tensor_mul(
                        qlat[:], lat_raw[:],
                        rstd_sb[:].unsqueeze(1).broadcast_to([128, 4, 512]))
                else:
                    nc.vector.tensor_mul(
                        kvl_sb[:, :, 0:nf], lat_raw[:, :, 0:nf],
                        rstd_sb[:, 0:nf].unsqueeze(1).broadcast_to([128, 4, nf]))
                    # ship this half: latent + krope -> DRAM -> AllGather.
                    # Stores ride the same in-order queue as the input loads
                    # so deferrable prefetches (emitted after) cannot take
                    # the serialized DMA device ahead of them.
                    t = sup
                    nc.sync.dma_start(
                        lat_kin[t][0:512, :].rearrange("(c p) n -> p c n", p=128),
                        kvl_sb[:, :, 0:nf])
                    nc.sync.dma_start(lat_kin[t][512:576, :], kr_sb[:, xs])
                    if t == 0:
                        nc.sync.dma_start(wdqn_a, w_dqn[:, :, 0:HPP * 128])
                        nc.sync.dma_start(wdkn_t[:], w_dkn[:])
                        nc.sync.dma_start(mask_t[:], masks[:])
                    nc.gpsimd.collective_compute(
                        "AllGather",
                        mybir.AluOpType.bypass,
                        ins=[lat_kin[t][:].rearrange("a b -> (a b)")],
                        outs=[lat_g[t][:].rearrange("w a b -> (w a b)")],
                        replica_groups=[[g * 4 + i for i in range(4)]
                                        for g in range(2)],
                    )


            # load gathered half 0 (gathered order; attention uses GPERM).
            # Half 1 is emitted inside pass 0 so its AG1-gated wait doesn't
            # coarsen into half-0 consumers.
            def load_gathered(t):
                # first 512 keys (cols 0:512 = gathered blocks 0-1) of every
                # latent chunk land first so kvdec's ksl=0 can start early
                for cs_ in (slice(0, 512), slice(512, 1024)):
                    for c in range(4):
                        nc.gpsimd.dma_start(
                            kvlat_h[t][:, c, cs_].rearrange(
                                "p (w m) -> p w m", w=2),
                            lat_g[t][cs_.start // 256:cs_.stop // 256,
                                     c * 128:(c + 1) * 128, :]
                            .rearrange("w p m -> p w m"))
                nc.gpsimd.dma_start(
                    krope_h[t][:].rearrange("p (w m) -> p w m", w=4),
                    lat_g[t][:, 512:576, :].rearrange("w p m -> p w m"))

            load_gathered(0)

        # v decompress weights for both passes: pass-0 half issued here so it
        # lands during qdec; pass-1 half goes out during pass 0
        dw_cm = tc.tile_pool(name="dec_w", bufs=1)
        dec_w = dw_cm.__enter__()
        wdv_t = dec_w.tile([128, 4, H * 128], FP16, name="wdv_t")
        nc.sync.dma_start(wdv_t[:, :, 0:HPP * 128], w_dv[:, :, 0:HPP * 128])

        # remaining qdec weights: issued right after compress, land mid-qdec
        qdw2_cm = tc.tile_pool(name="qdec_w2", bufs=1)
        qdec_w2 = qdw2_cm.__enter__()
        wdqr2_t = qdec_w2.tile([128, 4, H * 128], FP16, name="wdqr2_t")
        cs_q2_t = qdec_w2.tile([128, LQ], FP16, name="cs_q2_t")
        msc_q2_t = qdec_w2.tile([128, LQ], FP16, name="msc_q2_t")
        nc.scalar.dma_start(wdqn_b, w_dqn[:, :, HPP * 128:H * 128])
        nc.scalar.dma_start(wdqr2_t[:], w_dqr2[:])
        nc.sync.dma_start(cs_q2_t[:], cs_q2[:])
        nc.sync.dma_start(msc_q2_t[:], msc_q2[:])

        # q decompress for all 16 heads up front (only needs qlat; fills the
        # PE while the latent AllGathers are in flight)
        with tc.tile_pool(name="qdec_t", bufs=2) as qdec_t, \
             tc.tile_pool(name="ps_qd", bufs=3, space="PSUM") as ps_qd:
            # heads 0-11; heads 12-15 (pass-1 consumers) decompress at
            # pass-0 s==4, filling the PE while AllGather-1 lands. Heads
            # 8-11 sit here to cover the AllGather-0 wait before kvdec.
            for h in range(HPP + 4):
                ps = ps_qd.tile([128, 512], FP32, name="ps_qn", tag="ps_q")
                if h < HPP:
                    contract(ps[:], wdqn_a, 0, slice(h * 128, (h + 1) * 128),
                             qlat, 0, slice(0, LQ), 4)
                else:
                    contract(ps[:], wdqn_b, 0,
                             slice((h - HPP) * 128, (h - HPP + 1) * 128),
                             qlat, 0, slice(0, LQ), 4)
                cp(qn_all[:, h, :], ps[:])
            for pr in range(H // 2):
                psp = ps_qd.tile([128, 512], FP32, name="ps_p2", tag="ps_q")
                psr = ps_qd.tile([128, 512], FP32, name="ps_r2", tag="ps_q")
                contract(psp[:], wdqr2_t, 0,
                         slice(pr * 256, pr * 256 + 128),
                         qlat, 0, slice(0, LQ), 4)
                contract(psr[:], wdqr2_t, 0,
                         slice(pr * 256 + 128, pr * 256 + 256),
                         qlat, 0, slice(0, LQ), 4)
                u2 = qdec_t.tile([128, 512], FP16, name="qu2", tag="qu2")
                v2 = qdec_t.tile([128, 512], FP16, name="qv2", tag="qv2")
                qtmp = qdec_t.tile([128, 512], FP16, name="qtmp", tag="qtmp")
                nc.vector.tensor_mul(u2[:], psp[:], cs_q2_t[:])
                nc.vector.tensor_mul(v2[:], psr[:], msc_q2_t[:])
                # head 2pr combo rows sit in partitions 0:64, head 2pr+1 in
                # 64:128; the odd head needs a partition-shifting DMA hop.
                nc.vector.tensor_add(qr2b[0:64, 2 * pr, :],
                                     u2[0:64, :], v2[0:64, :])
                nc.vector.tensor_add(qtmp[64:128, :],
                                     u2[64:128, :], v2[64:128, :])
                nc.scalar.dma_start(qr2b[0:64, 2 * pr + 1, :], qtmp[64:128, :])
        qdw2_cm.__exit__(None, None, None)

        # ---------------- Phase B: per head-pass decompress + attention ------
        for hp in range(NPASS):
            hbase = hp * HPP
            with tc.tile_pool(name="kv_sb", bufs=1) as kv_sb:
                psd_cm = tc.tile_pool(name="ps_dec", bufs=2, space="PSUM")
                ps_dec = psd_cm.__enter__()

                # --- kv decompress (8 heads, all 2048 keys) ---
                k_n = kv_sb.tile([128, HPP, S], FP16, name="k_n")
                v_t = kv_sb.tile([128, NKT, HPP * VH], FP16, name="v_t")

                def kvdec_half(tg):
                    lat = kvlat_h[tg]
                    for ksl in range(2):
                        ks = slice(ksl * 512, (ksl + 1) * 512)
                        ksg = slice((2 * tg + ksl) * 512, (2 * tg + ksl + 1) * 512)
                        for h in range(HPP):
                            ps = ps_dec.tile([128, 512], FP32, name="ps_kn",
                                             tag="ps_d")
                            contract(ps[:], wdkn_t, 0,
                                     slice((hbase + h) * 128,
                                           (hbase + h + 1) * 128),
                                     lat, 0, ks, 4)
                            cp(k_n[:, h, ksg], ps[:])
                    for ktl in range(8):
                        kt = 8 * tg + ktl
                        for g in range(2):      # head groups of 4
                            ps = ps_dec.tile([128, 512], FP32, name="ps_v",
                                             tag="ps_d")
                            for i in range(4):
                                nc.tensor.matmul(
                                    ps[:], lat[:, i, ktl * 128:(ktl + 1) * 128],
                                    wdv_t[:, i, hbase * 128 + g * 512:
                                          hbase * 128 + (g + 1) * 512],
                                    start=(i == 0), stop=(i == 3))
                            cp(v_t[:, kt, g * 512:(g + 1) * 512], ps[:])

                if hp == 0:
                    nc.sync.dma_start(wdv_t[:, :, HPP * 128:H * 128],
                                      w_dv[:, :, HPP * 128:H * 128])
                kvdec_half(0)
                ps_att_cm = tc.tile_pool(name="ps_att", bufs=2, space="PSUM")
                ps_att = ps_att_cm.__enter__()
                ps_avz_cm = tc.tile_pool(name="ps_avz", bufs=2, space="PSUM")
                ps_avz = ps_avz_cm.__enter__()
                att_cm = tc.tile_pool(name="att_t", bufs=2)
                att_t = att_cm.__enter__()

                # --- attention: 8 slots, slot s has 2s+2 key tiles ---
                # pav accumulation groups at F-offsets of one psum bank must
                # be sequential per head (interleaving breaks psum), so keep
                # the whole slot's exp'd probabilities in SBUF, then run each
                # head's AV as one contiguous accumulation group.
                def emit_av(st):
                    s_, pav_, pz_, pTs_ = st
                    trip_ = 2 * s_ + 2
                    qs_ = slice(s_ * QW, (s_ + 1) * QW)
                    for h in range(HPP):
                        fs = slice(h * QW, (h + 1) * QW)
                        vs = slice(h * VH, (h + 1) * VH)
                        for r in range(trip_):
                            nc.tensor.matmul(
                                pav_[:, fs], v_t[:, GPERM[r], vs],
                                pTs_[:, r, fs],
                                start=(r == 0), stop=(r == trip_ - 1))
                    rz = att_t.tile([128, 512], FP32, name="rz", tag="rz",
                                    bufs=1)
                    nc.vector.reciprocal(rz[:], pz_[:])
                    nc.vector.tensor_mul(
                        attn_out[:, hbase:hbase + HPP, qs_],
                        pav_[:].rearrange("p (h q) -> p h q", h=HPP),
                        rz[:].rearrange("p (h q) -> p h q", h=HPP))

                pend = None
                for s in range(NSLOT):
                    if s == 4:
                        if hp == 0:
                            # deferred q decompress (heads 12-15): useful PE
                            # work while AllGather-1 completes
                            for h in range(HPP + 4, H):
                                ps = ps_dec.tile([128, 512], FP32,
                                                 name="ps_qn2", tag="ps_d")
                                contract(ps[:], wdqn_b, 0,
                                         slice((h - HPP) * 128,
                                               (h - HPP + 1) * 128),
                                         qlat, 0, slice(0, LQ), 4)
                                cp(qn_all[:, h, :], ps[:])
                            load_gathered(1)
                        kvdec_half(1)
                    trip = 2 * s + 2
                    qs = slice(s * QW, (s + 1) * QW)
                    pav = ps_avz.tile([128, 512], FP32, name="pav", tag="pav")
                    pz = ps_avz.tile([128, 512], FP32, name="pz", tag="pz",
                                     bufs=1)
                    pTs = att_t.tile([128, NKT, 512], FP16,
                                     name="pTs", tag="pTs", bufs=2)
                    for r in range(trip):
                        g = GPERM[r]
                        ks = slice(g * 128, (g + 1) * 128)
                        sc = ps_att.tile([128, 512], FP32, name="sc", tag="sc",
                                         bufs=3)
                        # per-head nope then rope: accumulation groups must
                        # be contiguous (the PE holds one open group; a new
                        # start discards any open accumulation)
                        for h in range(HPP):
                            fs = slice(h * QW, (h + 1) * QW)
                            nc.tensor.matmul(sc[:, fs], k_n[:, h, ks],
                                             qn_all[:, hbase + h, qs],
                                             start=True, stop=False)
                            nc.tensor.matmul(
                                sc[:, fs],
                                krope_h[g // 8][:, (g % 8) * 128:
                                                (g % 8) * 128 + 128],
                                qr2b[0:64, hbase + h, qs],
                                start=False, stop=True)
                        if r >= 2 * s:      # masked iters (diag + pad)
                            m = 2 * s + (r - 2 * s)
                            scv = sc[:].rearrange("p (h q) -> p h q", h=HPP)
                            mb = mask_t[:, m, :].unsqueeze(1).broadcast_to(
                                [128, HPP, QW])
                            nc.vector.tensor_add(scv, scv, mb)
                        nc.scalar.activation(pTs[:, r, :], sc[:], AF.Exp,
                                             bias=zero1[:])
                        if r == 1 and pend is not None:
                            emit_av(pend)
                            pend = None
                        # pz accumulates the full bank (safe to interleave
                        # with sc groups); deferred two iters so the PE never
                        # waits on in-flight exps.
                        if r > 1:
                            nc.tensor.matmul(pz[:], ones_h[:], pTs[:, r - 2, :],
                                             start=(r == 2), stop=False)
                    nc.tensor.matmul(pz[:], ones_h[:], pTs[:, trip - 2, :],
                                     start=(trip == 2), stop=False)
                    nc.tensor.matmul(pz[:], ones_h[:], pTs[:, trip - 1, :],
                                     start=False, stop=True)
                    pend = (s, pav, pz, pTs)
                if pend is not None:
                    emit_av(pend)
                    pend = None
                att_cm.__exit__(None, None, None)
                ps_avz_cm.__exit__(None, None, None)
                ps_att_cm.__exit__(None, None, None)
                psd_cm.__exit__(None, None, None)

        dw_cm.__exit__(None, None, None)

        if DBG:
            nc.sync.dma_start(dbg_qr2b[:], qr2b[:])
            nc.sync.dma_start(dbg_qn[:], qn_all[:])
            nc.sync.dma_start(dbg_qlat[:], qlat[:])
            nc.sync.dma_start(dbg_kvl[:], kvlat_h[0][:])
            nc.sync.dma_start(dbg_ao[:], attn_out[:])

        # ---------------- Phase C: output projection -------------------------
        with tc.tile_pool(name="prj_w", bufs=3) as prj_w, \
             tc.tile_pool(name="prj_t", bufs=3) as prj_t, \
             tc.tile_pool(name="ps_prj", bufs=3, space="PSUM") as ps_prj:
            for ob in range(16):
                wp = prj_w.tile([128, 16, 128], FP16, name="wp", tag="wp")
                # w_proj is prepped ob-major: [:, ob, :] is one contiguous
                # 4KB/partition run (cheap descriptors)
                nc.gpsimd.dma_start(wp[:].rearrange("p a b -> p (a b)"),
                                    w_proj[:, ob, :])
                ps = ps_prj.tile([128, 512], FP32, name="ps_o", tag="ps_o")
                for h in range(16):
                    nc.tensor.matmul(ps[:], wp[:, h, :], attn_out[:, h, :],
                                     start=(h == 0), stop=(h == 15))
                ot = prj_t.tile([128, 512], FP32, name="ot", tag="ot")
                cp(ot[:], ps[:])
                nc.scalar.dma_start(out_c[:, ob, :], ot[:])

        ap_cm.__exit__(None, None, None)
        lat_cm.__exit__(None, None, None)
        dram_cm.__exit__(None, None, None)
        const_cm.__exit__(None, None, None)

    nc.compile()
    return nc


def _qsel(j):
    """Local q token order for core with within-batch index j."""
    idx = []
    for s in range(NSLOT):
        t = 4 * s + 3 - j
        idx.extend(range(t * QW, (t + 1) * QW))
    return np.array(idx)


def _prep_inputs(x, freqs_cis, w_cq, w_qnorm, w_dqn, w_dqr, w_ckv, w_kvnorm,
                 w_dkn, w_dv, w_krope, w_proj):
    f16 = np.float16

    perm = np.concatenate([np.arange(0, ROPE, 2), np.arange(1, ROPE, 2)])
    pe, po = perm[:HALF], perm[HALF:]

    def chunk_major(a, nch):
        # [K, C] -> [128, nch, C] with K = 128*nch
        return np.ascontiguousarray(
            a.reshape(nch, 128, a.shape[1]).transpose(1, 0, 2))

    # compress weights (lhsT layout [K=D, P=out])
    wcq_l = chunk_major(w_cq.T.astype(f16), 16)                   # [128,16,512]
    wkr = (w_krope / H)                                           # [64, D]
    ckx = np.concatenate([w_ckv, wkr[pe], wkr[pe], wkr[po], wkr[po]], axis=0)
    # krope block rows 512:640: p2 = [even;even], r2 = [odd;odd]
    wckx_l = chunk_major(ckx.T.astype(f16), 16)                   # [128,16,640]

    # decompress weights, norm + scale folded
    dqn = (w_dqn * w_qnorm[None, :] * SCALE)                      # [H*128, QR]
    wdqn_l = chunk_major(dqn.T.astype(f16), 4)                    # [128,4,2048]
    dqr = (w_dqr * w_qnorm[None, :] * SCALE).reshape(H, ROPE, QR)
    dqr2 = np.empty((H // 2, 4, HALF * 2, QR), np.float32)
    for p in range(H // 2):
        h0, h1 = 2 * p, 2 * p + 1
        # rows: [x0;x0] for p2 blocks, [x1;x1] for r2 blocks
        dqr2[p, 0, :HALF] = dqr[h0][pe]; dqr2[p, 0, HALF:] = dqr[h0][pe]
        dqr2[p, 1, :HALF] = dqr[h1][pe]; dqr2[p, 1, HALF:] = dqr[h1][pe]
        dqr2[p, 2, :HALF] = dqr[h0][po]; dqr2[p, 2, HALF:] = dqr[h0][po]
        dqr2[p, 3, :HALF] = dqr[h1][po]; dqr2[p, 3, HALF:] = dqr[h1][po]
    # layout per pair: cols [p2_h0(64) p2_h1(64) r2_h0(64) r2_h1(64)]
    dqr2 = dqr2.reshape(H // 2 * 4 * ROPE, QR)                    # [2048, 512]
    wdqr2_l = chunk_major(np.ascontiguousarray(dqr2.T).astype(f16), 4)
    dkn = (w_dkn * w_kvnorm[None, :])
    wdkn_l = chunk_major(dkn.T.astype(f16), 4)
    dvw = (w_dv * w_kvnorm[None, :])
    wdv_l = chunk_major(dvw.T.astype(f16), 4)
    # ob-major proj layout: [:, ob, :] = the 16 K-chunks x 128 d-cols of
    # output block ob, contiguous per partition
    wproj_l = chunk_major(np.ascontiguousarray(w_proj.T).astype(f16), 16)
    wproj_l = np.ascontiguousarray(
        wproj_l.reshape(128, 16, 16, 128).transpose(0, 2, 1, 3)
    ).reshape(128, 16, 2048)

    cos = freqs_cis[:, :, 0].T.astype(np.float32)                 # [32, S]
    sin = freqs_cis[:, :, 1].T.astype(np.float32)
    cs_kv = np.concatenate([cos, sin], 0).astype(f16)             # [64, S]
    msc_kv = np.concatenate([-sin, cos], 0).astype(f16)

    xT = [np.ascontiguousarray(x[b].T) for b in range(B)]         # [D, S]

    in_maps = []
    for c in range(W):
        b, j = c // 4, c % 4
        qsel = _qsel(j)
        kvsel = np.concatenate([np.arange(128 * (4 * k + j), 128 * (4 * k + j) + 128)
                                for k in range(4)])
        xkv_l = chunk_major(np.ascontiguousarray(xT[b][:, kvsel]).astype(f16), 16)
        xq_l = chunk_major(np.ascontiguousarray(xT[b][:, qsel]).astype(f16), 16)
        csq = cs_kv[:, qsel]
        mscq = msc_kv[:, qsel]
        cs_q2 = np.concatenate([csq, csq], 0)                     # [128, LQ]
        msc_q2 = np.concatenate([mscq, mscq], 0)
        # masks: slot s, d in {0,1} -> iter r = 2s+d, additive 0/-10000
        mk = np.zeros((128, 16, QW), np.float32)
        for s in range(NSLOT):
            for d_ in range(2):
                r = 2 * s + d_
                kg = r * 128 + np.arange(128)
                qg = qsel[s * QW:(s + 1) * QW]
                mk[:, 2 * s + d_, :] = np.where(qg[None, :] >= kg[:, None],
                                                0.0, -10000.0)
        in_maps.append({
            "x_kv": xkv_l, "x_q": xq_l,
            "w_cq": wcq_l, "w_ckx": wckx_l,
            "w_dqn": wdqn_l, "w_dqr2": wdqr2_l, "w_dkn": wdkn_l, "w_dv": wdv_l,
            "w_proj": wproj_l,
            "cs_kv": np.ascontiguousarray(cs_kv[:, kvsel]),
            "msc_kv": np.ascontiguousarray(msc_kv[:, kvsel]),
            "cs_q2": cs_q2.astype(f16), "msc_q2": msc_q2.astype(f16),
            "masks": mk.astype(f16),
        })
    return in_maps


last_results = None


def kernel(x, mask, freqs_cis, w_cq, w_qnorm, w_dqn, w_dqr, w_ckv, w_kvnorm,
           w_dkn, w_dv, w_krope, w_proj):
    global last_results
    if "nc" not in _cache:
        _cache["nc"] = _build()
    nc = _cache["nc"]

    args = [np.asarray(a, np.float32) for a in
            (x, freqs_cis, w_cq, w_qnorm, w_dqn, w_dqr, w_ckv, w_kvnorm,
             w_dkn, w_dv, w_krope, w_proj)]
    in_maps = _prep_inputs(*args)

    res = bass_utils.run_bass_kernel_spmd(nc, in_maps, core_ids=list(range(W)))
    last_results = res

    out = np.empty((B, S, D), np.float32)
    for c in range(W):
        b, j = c // 4, c % 4
        oc = res.results[c]["out_c"]          # [128, 16, 512]
        flat = oc.transpose(1, 0, 2).reshape(D, LQ)
        out[b, _qsel(j), :] = flat.T
    return out
